# revision 1
# baseline (speedup 1.0000x reference)
"""DCGRU cell (nn_DCGRUCell) Trainium2 Bass kernel, 8 NeuronCores.

Sharding: node dimension N=4096 split 8 ways (512 rows/core); supports are
fed host-transposed (T = A^T) so tensor-engine matmuls need no on-device
transposes of A. Hop-1 diffusion products are computed node-major
[node, (batch, feat)] and AllGathered across cores; hop-2 products are
computed directly in transposed (feature-major) form since they only feed
the dense W stage. All matmuls run fp32 (float32r streaming); PSUM fp32.

kernel(**inputs) takes the FULL inputs from reference.setup_inputs() and
returns the FULL [16, 4096, 64] float32 output.
"""
import os
import numpy as np

import concourse.bass as bass
import concourse.mybir as mybir
import concourse.tile as tile
from concourse import bacc
from concourse.bass_utils import run_bass_kernel_spmd

F32 = mybir.dt.float32
F32R = mybir.dt.float32r
AF = mybir.ActivationFunctionType

NCORES = 8
B, N, H, DIN = 16, 4096, 64, 2
C = DIN + H                 # 66 features per batch into each GCN
BC = B * C                  # 1056
NOWN = N // NCORES          # 512 rows per core
NT = NOWN // 128            # 4 n-tiles per core
MT = N // 128               # 32 m-tiles (contraction)
MAIN = 1024                 # bc columns in the node-major main sweep
RAG = BC - MAIN             # 32 ragged columns
JT = BC // 128              # 8 full 128-col feature tiles (+1 ragged)
MAIN_ELEMS = NOWN * MAIN
RAG_ELEMS = NT * 128 * RAG
SHARD = MAIN_ELEMS + RAG_ELEMS
GROUP = [list(range(NCORES))]

_NC_CACHE = {}


def build_nc():
    nc = bacc.Bacc("TRN2", target_bir_lowering=False, debug=False,
                   num_devices=NCORES)

    d = {}
    d["Ts"] = nc.dram_tensor("Ts", [2, N, NOWN], F32R, kind="ExternalInput")
    d["xs_main"] = nc.dram_tensor("xs_main", [N, MAIN], F32R,
                                  kind="ExternalInput")
    d["xs_rag"] = nc.dram_tensor("xs_rag", [MT, 128, RAG], F32R,
                                 kind="ExternalInput")
    d["xsT_orig"] = nc.dram_tensor("xsT_orig", [BC, NOWN], F32R,
                                   kind="ExternalInput")
    d["xsT_own"] = nc.dram_tensor("xsT_own", [BC, NOWN], F32R,
                                  kind="ExternalInput")
    d["Wg"] = nc.dram_tensor("Wg", [5 * C, 2 * H], F32R, kind="ExternalInput")
    d["bg"] = nc.dram_tensor("bg", [2 * H, 1], F32, kind="ExternalInput")
    d["Wu"] = nc.dram_tensor("Wu", [5 * C, H], F32R, kind="ExternalInput")
    d["bu"] = nc.dram_tensor("bu", [H, 1], F32, kind="ExternalInput")
    d["negI"] = nc.dram_tensor("negI", [128, 128], F32R, kind="ExternalInput")
    d["outT"] = nc.dram_tensor("outT", [B, H, NOWN], F32,
                               kind="ExternalOutput")

    with tile.TileContext(nc) as tc:
        _emit(nc, tc, d)
    nc.compile()
    return nc


def _emit(nc, tc, d):
    import contextlib
    stack = contextlib.ExitStack()
    with stack:
        const = stack.enter_context(tc.tile_pool(name="const", bufs=1))
        sb_ex = stack.enter_context(tc.tile_pool(name="ex", bufs=1))
        sb_mov = stack.enter_context(tc.tile_pool(name="mov", bufs=1))
        sb_sm = stack.enter_context(tc.tile_pool(name="small", bufs=1))
        dram = stack.enter_context(
            tc.tile_pool(name="dram", bufs=1, space="DRAM"))
        psum = stack.enter_context(
            tc.tile_pool(name="psum", bufs=1, space="PSUM"))

        # ---- constants / resident tensors ----
        # supports loaded in interleaved 4-m-tile chunks so the first
        # matmuls only wait for the first small chunk
        CH = 4
        NCH = MT // CH
        Tch = {}
        for s in range(2):
            for k in range(NCH):
                Tch[(s, k)] = const.tile([128, CH, 512], F32R,
                                         name=f"T{s}_{k}")
        for k in range(NCH):
            for s in range(2):
                ts = d["Ts"].ap()[s].rearrange("(t p) n -> p t n", p=128)
                nc.sync.dma_start(Tch[(s, k)][:],
                                  ts[:, k * CH:(k + 1) * CH, :])

        def T_tile(s, m):
            return Tch[(s, m // CH)][:, m % CH, :]

        ident = const.tile([128, 128], F32)
        nc.gpsimd.memset(ident[:], 0.0)
        nc.gpsimd.affine_select(
            out=ident[:], in_=ident[:],
            compare_op=mybir.AluOpType.not_equal, fill=1.0, base=0,
            pattern=[[-1, 128]], channel_multiplier=1)
        nident = const.tile([128, 128], F32R)  # -0.5*I (for 2*(Ax - X/2))
        nc.sync.dma_start(nident[:], d["negI"].ap())

        wg_t = const.tile([C, 5, 2 * H], F32R)
        wu_t = const.tile([C, 5, H], F32R)
        for j in range(5):
            nc.sync.dma_start(wg_t[:, j, :],
                              d["Wg"].ap()[j * C:(j + 1) * C, :])
            nc.sync.dma_start(wu_t[:, j, :],
                              d["Wu"].ap()[j * C:(j + 1) * C, :])
        bg_t = const.tile([2 * H, 1], F32)
        nc.sync.dma_start(bg_t[:], d["bg"].ap())
        bu_t = const.tile([H, 1], F32)
        nc.sync.dma_start(bu_t[:], d["bu"].ap())

        # ---- DRAM staging ----
        # AG slots: 0,1 = y1 of gcn1; 2 = cand; 3,4 = y1 of gcn2
        ag_in = [dram.tile([SHARD], F32R, name=f"agin{i}") for i in range(5)]
        ag_out = [dram.tile([NCORES * SHARD], F32R, name=f"agout{i}",
                            addr_space="Shared") for i in range(5)]
        yt = [dram.tile([BC, NOWN], F32R, name=f"yt{i}") for i in range(4)]
        yt2 = [dram.tile([BC, NOWN], F32R, name=f"yt2_{i}") for i in range(4)]
        candT_dram = dram.tile([BC, NOWN], F32R)
        rt_dram = dram.tile([B, H, NOWN], F32)

        def xs_main_half(m, h):
            return d["xs_main"].ap()[m * 128:(m + 1) * 128,
                                     h * 512:(h + 1) * 512]

        def xs_rag_tile(m):
            return d["xs_rag"].ap()[m]

        def ag_main_half(i, m, h):
            a = ag_out[i].opt()
            off = (m // NT) * SHARD + (m % NT) * 128 * MAIN
            v = a[off:off + 128 * MAIN].rearrange("(p f) -> p f", f=MAIN)
            return v[:, h * 512:(h + 1) * 512]

        def ag_rag_tile(i, m):
            a = ag_out[i].opt()
            off = (m // NT) * SHARD + MAIN_ELEMS + (m % NT) * 128 * RAG
            return a[off:off + 128 * RAG].rearrange("(p f) -> p f", f=RAG)

        def ag_col_chunk(i, q, j):
            """[128, NT, 128] moving chunk: rank q's own rows, feature
            columns 128j..128(j+1)."""
            a = ag_out[i].opt()
            blk = a[q * SHARD:q * SHARD + MAIN_ELEMS].rearrange(
                "(t p f) -> p t f", p=128, f=MAIN)
            return blk[:, :, j * 128:(j + 1) * 128]

        def agin_own_main(i, t):
            a = ag_in[i].opt()
            return a[t * 128 * MAIN:(t + 1) * 128 * MAIN].rearrange(
                "(p f) -> p f", f=MAIN)

        def agin_own_main_half(i, t, h):
            return agin_own_main(i, t)[:, h * 512:(h + 1) * 512]

        def agin_own_rag(i, t):
            a = ag_in[i].opt()
            off = MAIN_ELEMS + t * 128 * RAG
            return a[off:off + 128 * RAG].rearrange("(p f) -> p f", f=RAG)

        # ============ hop-1 products: node-major + AllGather ============
        # Both supports share one pass over the moving operand.
        def emit_hop1_pair(pid, mov_main, mov_rag, agin_idx, yt_dst):
            """Y1_s[own rows, :] = A_s @ M for s in (0, 1)."""
            # ragged columns, transposed: psum[c(32), n(512)] per support
            ps_t = [psum.tile([RAG, NOWN], F32, name=f"pst{pid}{s}",
                              tag="acc", bufs=8) for s in range(2)]
            for m in range(MT):
                mvr = sb_mov.tile([128, RAG], F32R, name=f"mvr{pid}_{m}",
                                  tag="movr", bufs=8)
                nc.sync.dma_start(mvr[:], mov_rag(m))
                for s in range(2):
                    nc.tensor.matmul(ps_t[s][:], mvr[:], T_tile(s, m),
                                     start=(m == 0), stop=(m == MT - 1))
            for s in range(2):
                rag_ex = sb_sm.tile([RAG, NOWN], F32, name=f"rgex{pid}{s}",
                                    tag="ragex", bufs=1)
                nc.vector.tensor_copy(rag_ex[:], ps_t[s][:])
                nc.sync.dma_start(yt_dst[s].opt()[MAIN:BC, :].bitcast(F32),
                                  rag_ex[:])
                for t in range(NT):
                    tp = psum.tile([128, RAG], F32, name=f"rtp{pid}{s}",
                                   tag="acc", bufs=8)
                    nc.tensor.transpose(
                        tp[:], rag_ex[:, t * 128:(t + 1) * 128],
                        ident[0:RAG, 0:RAG])
                    rnm = sb_sm.tile([128, RAG], F32, name=f"rnm{pid}{s}",
                                     tag="rnm", bufs=2)
                    nc.vector.tensor_copy(rnm[:], tp[:])
                    nc.sync.dma_start(
                        agin_own_rag(agin_idx[s], t).bitcast(F32), rnm[:])

            # main columns in two 512-wide sweeps; 2 supports x 4 n-tiles
            # of accumulators fill all 8 PSUM banks per sweep
            for hh in range(2):
                ps_m = {}
                for s in range(2):
                    for n in range(NT):
                        ps_m[(s, n)] = psum.tile(
                            [128, 512], F32, name=f"psm{pid}_{hh}{s}{n}",
                            tag="acc", bufs=8)
                for m in range(MT):
                    mv = sb_mov.tile([128, 512], F32R,
                                     name=f"mv{pid}_{hh}_{m}", tag="mov",
                                     bufs=4)
                    nc.sync.dma_start(mv[:], mov_main(m, hh))
                    for s in range(2):
                        for n in range(NT):
                            nc.tensor.matmul(
                                ps_m[(s, n)][:],
                                T_tile(s, m)[:, n * 128:(n + 1) * 128],
                                mv[:], start=(m == 0), stop=(m == MT - 1))
                for s in range(2):
                    exhs = []
                    for n in range(NT):
                        exh = sb_ex.tile([128, 512], F32,
                                         name=f"ex{pid}{hh}{s}{n}",
                                         tag="ex", bufs=6)
                        nc.vector.tensor_copy(exh[:], ps_m[(s, n)][:])
                        nc.sync.dma_start(
                            agin_own_main_half(agin_idx[s], n, hh)
                            .bitcast(F32), exh[:])
                        exhs.append(exh)
                    # feature-major staging: per bc row-block j, transpose
                    # the 4 n-chunks and write one contiguous row-block
                    for j in range(4):
                        st4 = sb_sm.tile([128, NOWN], F32,
                                         name=f"st4{pid}", tag="st", bufs=2)
                        for n in range(NT):
                            tp = psum.tile([128, 128], F32,
                                           name=f"tp{pid}", tag="acc",
                                           bufs=8)
                            nc.tensor.transpose(
                                tp[:], exhs[n][:, j * 128:(j + 1) * 128],
                                ident[:])
                            nc.vector.tensor_copy(
                                st4[:, n * 128:(n + 1) * 128], tp[:])
                        jj = hh * 4 + j
                        nc.sync.dma_start(
                            yt_dst[s].opt()[jj * 128:(jj + 1) * 128, :]
                            .bitcast(F32), st4[:])
            nc.gpsimd.collective_compute(
                "AllGather", mybir.AluOpType.bypass, replica_groups=GROUP,
                ins=[ag_in[agin_idx[0]].opt()],
                outs=[ag_out[agin_idx[0]].opt()])
            nc.gpsimd.collective_compute(
                "AllGather", mybir.AluOpType.bypass, replica_groups=GROUP,
                ins=[ag_in[agin_idx[1]].opt()],
                outs=[ag_out[agin_idx[1]].opt()])

        # ======= hop-2 product: transposed form (feature-major out) =======
        def emit_hop2(pid, s, ag_idx, ownT_rows, yt_dst):
            """Y2^T[bc, own n] = 2*(A_s @ Y1)^T[bc, n] - X^T[bc, n].

            Moving operand = gathered Y1 (ag_out[ag_idx]) loaded as full
            m-rows; its 128-col slices act as lhsT for 8 concurrent
            feature-tile accumulators. ownT_rows(j, w) gives X^T rows for
            the -X term."""
            # ragged feature tile (j = JT), its own accumulation
            ps_r = psum.tile([RAG, NOWN], F32, name=f"ph2r{pid}", tag="acc",
                             bufs=8)
            for m in range(MT):
                mvr = sb_mov.tile([128, RAG], F32R, name=f"mvr{pid}_{m}",
                                  tag="movr", bufs=8)
                nc.sync.dma_start(mvr[:], ag_rag_tile(ag_idx, m))
                nc.tensor.matmul(ps_r[:], mvr[:], T_tile(s, m),
                                 start=(m == 0), stop=False)
            xrt = sb_mov.tile([RAG, NOWN], F32R, name=f"xrt{pid}r",
                              tag="xrt", bufs=2)
            nc.sync.dma_start(xrt[:], ownT_rows(JT, RAG))
            nc.tensor.matmul(ps_r[:], nident[0:RAG, 0:RAG], xrt[:],
                             start=False, stop=True)
            exr = sb_ex.tile([RAG, NOWN], F32, name=f"h2exr{pid}",
                             tag="ex", bufs=6)
            nc.scalar.mul(exr[:], ps_r[:], 2.0)
            nc.sync.dma_start(
                yt_dst.opt()[MAIN:BC, :].bitcast(F32), exr[:])

            # 8 full feature tiles, m-outer (row loads are contiguous)
            ps = [psum.tile([128, NOWN], F32, name=f"ph2{pid}_{j}",
                            tag="acc", bufs=8) for j in range(JT)]
            for m in range(MT):
                mrow = sb_mov.tile([128, MAIN], F32R, name=f"mr{pid}_{m}",
                                   tag="mov", bufs=4)
                for h in range(2):
                    nc.sync.dma_start(mrow[:, h * 512:(h + 1) * 512],
                                      ag_main_half(ag_idx, m, h))
                for j in range(JT):
                    nc.tensor.matmul(
                        ps[j][:], mrow[:, j * 128:(j + 1) * 128],
                        T_tile(s, m), start=(m == 0), stop=False)
            for j in range(JT):
                xrt = sb_mov.tile([128, NOWN], F32R, name=f"xrt{pid}_{j}",
                                  tag="xrt", bufs=2)
                nc.sync.dma_start(xrt[:], ownT_rows(j, 128))
                nc.tensor.matmul(ps[j][:], nident[:], xrt[:],
                                 start=False, stop=True)
                exh = sb_ex.tile([128, NOWN], F32, name=f"h2ex{pid}_{j}",
                                 tag="ex", bufs=6)
                nc.scalar.mul(exh[:], ps[j][:], 2.0)
                nc.sync.dma_start(
                    yt_dst.opt()[j * 128:(j + 1) * 128, :].bitcast(F32),
                    exh[:])

        # ======================= GCN 1 (gate) =======================
        emit_hop1_pair("g1h1", xs_main_half, xs_rag_tile, (0, 1),
                       (yt[0], yt[2]))

        def xsT_orig_rows(j, w):
            return d["xsT_orig"].ap()[j * 128:j * 128 + w, :]

        emit_hop2("g1s0h2", 0, 0, xsT_orig_rows, yt[1])
        emit_hop2("g1s1h2", 1, 1, xsT_orig_rows, yt[3])

        # gate W-stage + candidate build
        for b in range(B):
            xsT_b = sb_sm.tile([C, NOWN], F32R, name="xsTb", tag="xsTb",
                               bufs=2)
            nc.sync.dma_start(xsT_b[:],
                              d["xsT_own"].ap()[b * C:(b + 1) * C, :])
            blocks = [xsT_b]
            for j in range(4):
                bt = sb_sm.tile([C, NOWN], F32R, name=f"blk{j}",
                                tag=f"blk{j}", bufs=2)
                nc.sync.dma_start(bt[:], yt[j].opt()[b * C:(b + 1) * C, :])
                blocks.append(bt)
            zr_ps = psum.tile([2 * H, NOWN], F32, name="zrps", tag="acc", bufs=8)
            for j in range(5):
                nc.tensor.matmul(zr_ps[:], wg_t[:, j, :], blocks[j][:],
                                 start=(j == 0), stop=(j == 4))
            zr = sb_sm.tile([2 * H, NOWN], F32, name="zr", tag="zr", bufs=1)
            nc.scalar.activation(zr[:], zr_ps[:], AF.Sigmoid, bias=bg_t[:])
            nc.sync.dma_start(rt_dram.opt()[b], zr[H:2 * H, :])
            # candT_b rows are [z*state(64); x(2)] (host permutes W rows)
            cT = sb_sm.tile([C, NOWN], F32, name="cT", tag="cT", bufs=1)
            nc.vector.tensor_mul(cT[0:H, :], zr[0:H, :],
                                 xsT_b[0:H, :].bitcast(F32))
            nc.vector.tensor_copy(cT[H:C, :], xsT_b[H:C, :].bitcast(F32))
            nc.sync.dma_start(
                candT_dram.opt()[b * C:(b + 1) * C, :].bitcast(F32), cT[:])
            # cand node-major -> ag_in[2]
            a_main = ag_in[2].opt()[0:MAIN_ELEMS].rearrange(
                "(p f) -> p f", f=MAIN).bitcast(F32)
            for t in range(NT):
                ps = psum.tile([128, C], F32, name="ctps", tag="acc", bufs=8)
                nc.tensor.transpose(ps[:], cT[:, t * 128:(t + 1) * 128],
                                    ident[0:C, 0:C])
                ct_nm = sb_sm.tile([128, C], F32, name="ctnm", tag="ctnm", bufs=1)
                nc.vector.tensor_copy(ct_nm[:], ps[:])
                lo, hi = b * C, (b + 1) * C
                if hi <= MAIN:
                    nc.sync.dma_start(
                        a_main[t * 128:(t + 1) * 128, lo:hi], ct_nm[:])
                else:
                    cut = MAIN - lo
                    nc.sync.dma_start(
                        a_main[t * 128:(t + 1) * 128, lo:MAIN],
                        ct_nm[:, 0:cut])
                    nc.sync.dma_start(agin_own_rag(2, t).bitcast(F32),
                                      ct_nm[:, cut:C])
        nc.gpsimd.collective_compute(
            "AllGather", mybir.AluOpType.bypass, replica_groups=GROUP,
            ins=[ag_in[2].opt()], outs=[ag_out[2].opt()])

        # ======================= GCN 2 (update) =======================
        emit_hop1_pair("g2h1",
                       lambda m, h: ag_main_half(2, m, h),
                       lambda m: ag_rag_tile(2, m), (3, 4),
                       (yt2[0], yt2[2]))

        def candT_rows(j, w):
            return candT_dram.opt()[j * 128:j * 128 + w, :]

        emit_hop2("g2s0h2", 0, 3, candT_rows, yt2[1])
        emit_hop2("g2s1h2", 1, 4, candT_rows, yt2[3])

        # update W-stage + final combine
        for b in range(B):
            cT_b = sb_sm.tile([C, NOWN], F32R, name="cTb", tag="xsTb",
                              bufs=2)
            nc.sync.dma_start(cT_b[:],
                              candT_dram.opt()[b * C:(b + 1) * C, :])
            blocks = [cT_b]
            for j in range(4):
                bt = sb_sm.tile([C, NOWN], F32R, name=f"ublk{j}",
                                tag=f"blk{j}", bufs=2)
                nc.sync.dma_start(bt[:], yt2[j].opt()[b * C:(b + 1) * C, :])
                blocks.append(bt)
            hc_ps = psum.tile([H, NOWN], F32, name="hcps", tag="acc", bufs=8)
            for j in range(5):
                nc.tensor.matmul(hc_ps[:], wu_t[:, j, :], blocks[j][:],
                                 start=(j == 0), stop=(j == 4))
            hc = sb_sm.tile([H, NOWN], F32, name="hc", tag="zr", bufs=1)
            nc.scalar.activation(hc[:], hc_ps[:], AF.Tanh, bias=bu_t[:])

            # out = hc + r * (state - hc);  stateT = xsT_own rows [0:H]
            xsT_b = sb_sm.tile([C, NOWN], F32, name="xsTb2", tag="cT",
                               bufs=1)
            nc.sync.dma_start(
                xsT_b[:],
                d["xsT_own"].ap()[b * C:(b + 1) * C, :].bitcast(F32))
            rT = sb_sm.tile([H, NOWN], F32, name="rT", tag="rT", bufs=1)
            nc.sync.dma_start(rT[:], rt_dram.opt()[b])
            tmp = sb_sm.tile([H, NOWN], F32, name="tmp", tag="tmp", bufs=2)
            nc.vector.tensor_sub(tmp[:], xsT_b[0:H, :], hc[:])
            nc.vector.tensor_mul(tmp[:], rT[:], tmp[:])
            ot = sb_sm.tile([H, NOWN], F32, name="ot", tag="ot", bufs=2)
            nc.vector.tensor_add(ot[:], hc[:], tmp[:])
            nc.sync.dma_start(d["outT"].ap()[b], ot[:])


def prepare_in_maps(x, state, support0, support1, W_gate, b_gate,
                    W_update, b_update):
    xs = np.concatenate([x, state], axis=-1)          # [B, N, C]
    xs_nm = np.ascontiguousarray(
        xs.transpose(1, 0, 2).reshape(N, BC)).astype(np.float32)
    # feature-major input for W / elementwise uses [state(64); x(2)] rows
    sx_nm = np.ascontiguousarray(
        np.concatenate([state, x], axis=-1)
        .transpose(1, 0, 2).reshape(N, BC)).astype(np.float32)
    perm = np.r_[DIN:C, 0:DIN]                 # [x, state] -> [state, x]
    Wg_dev = np.ascontiguousarray(W_gate, dtype=np.float32).copy()
    Wg_dev[0:C] = Wg_dev[0:C][perm]            # only the X-block reads xsT
    Wu_dev = np.ascontiguousarray(W_update, dtype=np.float32).copy()
    for j in range(5):                         # all of cand's blocks permute
        Wu_dev[j * C:(j + 1) * C] = Wu_dev[j * C:(j + 1) * C][perm]
    xs_main = np.ascontiguousarray(xs_nm[:, :MAIN])
    xs_rag = np.ascontiguousarray(xs_nm[:, MAIN:]).reshape(MT, 128, RAG)
    bg = np.ascontiguousarray(b_gate, dtype=np.float32).reshape(2 * H, 1)
    bu = np.ascontiguousarray(b_update, dtype=np.float32).reshape(H, 1)
    negI = (-0.5 * np.eye(128, dtype=np.float32))

    in_maps = []
    for r in range(NCORES):
        n0 = r * NOWN
        sl = xs_nm[n0:n0 + NOWN]
        in_maps.append({
            "Ts": np.ascontiguousarray(
                np.stack([support0[n0:n0 + NOWN, :].T,
                          support1[n0:n0 + NOWN, :].T])).astype(np.float32),
            "xs_main": xs_main,
            "xs_rag": xs_rag,
            "xsT_orig": np.ascontiguousarray(sl.T),
            "xsT_own": np.ascontiguousarray(sx_nm[n0:n0 + NOWN].T),
            "Wg": Wg_dev, "bg": bg, "Wu": Wu_dev, "bu": bu,
            "negI": negI,
        })
    return in_maps


def assemble_output(results):
    out = np.empty((B, N, H), dtype=np.float32)
    for r in range(NCORES):
        n0 = r * NOWN
        out[:, n0:n0 + NOWN, :] = results[r]["outT"].transpose(0, 2, 1)
    return out


def get_nc():
    if "nc" not in _NC_CACHE:
        _NC_CACHE["nc"] = build_nc()
    return _NC_CACHE["nc"]


def kernel(x, state, support0, support1, W_gate, b_gate, W_update, b_update):
    nc = get_nc()
    in_maps = prepare_in_maps(x, state, support0, support1,
                              W_gate, b_gate, W_update, b_update)
    prev = os.environ.get("BASS_NEVER_TRACE")
    os.environ["BASS_NEVER_TRACE"] = "1"
    try:
        res = run_bass_kernel_spmd(nc, in_maps, list(range(NCORES)),
                                   trace=False)
    finally:
        if prev is None:
            os.environ.pop("BASS_NEVER_TRACE", None)
        else:
            os.environ["BASS_NEVER_TRACE"] = prev
    return assemble_output(res.results)



# revision 7
# speedup vs baseline: 1.2163x; 1.2163x over previous
"""DCGRU cell (nn_DCGRUCell) Trainium2 Bass kernel, 8 NeuronCores.

Sharding: node dimension N=4096 split 8 ways (512 rows/core); supports are
fed host-transposed (T = A^T) so tensor-engine matmuls need no on-device
transposes of A. Hop-1 diffusion products are computed node-major
[node, (batch, feat)] and AllGathered across cores; hop-2 products are
computed directly in transposed (feature-major) form since they only feed
the dense W stage. Matmul operands are bf16 (PSUM accumulates fp32); the
Chebyshev combination x2 = 2*A@x1 - x0 is folded into the dense W matrices
host-side (W0 -= W2+W4, W2 *= 2, W4 *= 2), so hop-2 stages raw A@y1.

kernel(**inputs) takes the FULL inputs from reference.setup_inputs() and
returns the FULL [16, 4096, 64] float32 output.
"""
import os
import numpy as np

import concourse.bass as bass
import concourse.mybir as mybir
import concourse.tile as tile
from concourse import bacc
from concourse.bass_utils import run_bass_kernel_spmd

F32 = mybir.dt.float32
BF16 = mybir.dt.bfloat16
AF = mybir.ActivationFunctionType

NCORES = 8
B, N, H, DIN = 16, 4096, 64, 2
C = DIN + H                 # 66 features per batch into each GCN
BC = B * C                  # 1056
NOWN = N // NCORES          # 512 rows per core
NT = NOWN // 128            # 4 n-tiles per core
MT = N // 128               # 32 m-tiles (contraction)
MAIN = 1024                 # bc columns in the node-major main sweep
RAG = BC - MAIN             # 32 ragged columns
JT = BC // 128              # 8 full 128-col feature tiles (+1 ragged)
MAIN_ELEMS = NOWN * MAIN
RAG_ELEMS = NT * 128 * RAG
SHARD = MAIN_ELEMS + RAG_ELEMS
GROUP = [list(range(NCORES))]

_NC_CACHE = {}


def build_nc():
    nc = bacc.Bacc("TRN2", target_bir_lowering=False, debug=False,
                   num_devices=NCORES)

    d = {}
    d["Ts"] = nc.dram_tensor("Ts", [2, N, NOWN], BF16, kind="ExternalInput")
    d["xs_main"] = nc.dram_tensor("xs_main", [N, MAIN], BF16,
                                  kind="ExternalInput")
    d["xs_rag"] = nc.dram_tensor("xs_rag", [MT, 128, RAG], BF16,
                                 kind="ExternalInput")
    d["xsT_own"] = nc.dram_tensor("xsT_own", [BC, NOWN], BF16,
                                  kind="ExternalInput")
    d["stateT"] = nc.dram_tensor("stateT", [B, H, NOWN], F32,
                                 kind="ExternalInput")
    d["Wg"] = nc.dram_tensor("Wg", [5 * C, 2 * H], BF16, kind="ExternalInput")
    d["bg"] = nc.dram_tensor("bg", [2 * H, 1], F32, kind="ExternalInput")
    d["Wu"] = nc.dram_tensor("Wu", [5 * C, H], BF16, kind="ExternalInput")
    d["bu"] = nc.dram_tensor("bu", [H, 1], F32, kind="ExternalInput")
    d["outT"] = nc.dram_tensor("outT", [B, H, NOWN], F32,
                               kind="ExternalOutput")

    with tile.TileContext(nc) as tc:
        _emit(nc, tc, d)
    nc.compile()
    return nc


def _emit(nc, tc, d):
    import contextlib
    stack = contextlib.ExitStack()
    with stack:
        const = stack.enter_context(tc.tile_pool(name="const", bufs=1))
        sb_ex = stack.enter_context(tc.tile_pool(name="ex", bufs=1))
        sb_mov = stack.enter_context(tc.tile_pool(name="mov", bufs=1))
        sb_sm = stack.enter_context(tc.tile_pool(name="small", bufs=1))
        dram = stack.enter_context(
            tc.tile_pool(name="dram", bufs=1, space="DRAM"))
        psum = stack.enter_context(
            tc.tile_pool(name="psum", bufs=1, space="PSUM"))

        # ---- constants / resident tensors ----
        # supports loaded in interleaved 4-m-tile chunks so the first
        # matmuls only wait for the first small chunk
        CH = 4
        NCH = MT // CH
        Tch = {}
        for s in range(2):
            for k in range(NCH):
                Tch[(s, k)] = const.tile([128, CH, 512], BF16,
                                         name=f"T{s}_{k}")
        for k in range(NCH):
            for s in range(2):
                ts = d["Ts"].ap()[s].rearrange("(t p) n -> p t n", p=128)
                nc.sync.dma_start(Tch[(s, k)][:],
                                  ts[:, k * CH:(k + 1) * CH, :])

        def T_tile(s, m):
            return Tch[(s, m // CH)][:, m % CH, :]

        ident = const.tile([128, 128], F32)
        nc.gpsimd.memset(ident[:], 0.0)
        nc.gpsimd.affine_select(
            out=ident[:], in_=ident[:],
            compare_op=mybir.AluOpType.not_equal, fill=1.0, base=0,
            pattern=[[-1, 128]], channel_multiplier=1)
        identb = const.tile([128, 128], BF16)
        nc.vector.tensor_copy(identb[:], ident[:])

        wg_t = const.tile([C, 5, 2 * H], BF16)
        wu_t = const.tile([C, 5, H], BF16)
        for j in range(5):
            nc.sync.dma_start(wg_t[:, j, :],
                              d["Wg"].ap()[j * C:(j + 1) * C, :])
            nc.sync.dma_start(wu_t[:, j, :],
                              d["Wu"].ap()[j * C:(j + 1) * C, :])
        bg_t = const.tile([2 * H, 1], F32)
        nc.sync.dma_start(bg_t[:], d["bg"].ap())
        bu_t = const.tile([H, 1], F32)
        nc.sync.dma_start(bu_t[:], d["bu"].ap())

        # ---- DRAM staging ----
        # AG slots: 0,1 = y1 of gcn1; 2 = cand; 3,4 = y1 of gcn2
        ag_in = [dram.tile([SHARD], BF16, name=f"agin{i}") for i in range(5)]
        ag_out = [dram.tile([NCORES * SHARD], BF16, name=f"agout{i}",
                            addr_space="Shared") for i in range(5)]
        yt = [dram.tile([BC, NOWN], BF16, name=f"yt{i}") for i in range(4)]
        yt2 = [dram.tile([BC, NOWN], BF16, name=f"yt2_{i}") for i in range(4)]
        candT_dram = dram.tile([BC, NOWN], BF16)
        rt_dram = dram.tile([B, H, NOWN], F32)

        def xs_main_half(m, h):
            return d["xs_main"].ap()[m * 128:(m + 1) * 128,
                                     h * 512:(h + 1) * 512]

        def xs_rag_tile(m):
            return d["xs_rag"].ap()[m]

        def ag_main_half(i, m, h):
            a = ag_out[i].opt()
            off = (m // NT) * SHARD + (m % NT) * 128 * MAIN
            v = a[off:off + 128 * MAIN].rearrange("(p f) -> p f", f=MAIN)
            return v[:, h * 512:(h + 1) * 512]

        def ag_rag_tile(i, m):
            a = ag_out[i].opt()
            off = (m // NT) * SHARD + MAIN_ELEMS + (m % NT) * 128 * RAG
            return a[off:off + 128 * RAG].rearrange("(p f) -> p f", f=RAG)

        def agin_own_main(i, t):
            a = ag_in[i].opt()
            return a[t * 128 * MAIN:(t + 1) * 128 * MAIN].rearrange(
                "(p f) -> p f", f=MAIN)

        def agin_own_main_half(i, t, h):
            return agin_own_main(i, t)[:, h * 512:(h + 1) * 512]

        def agin_own_rag(i, t):
            a = ag_in[i].opt()
            off = MAIN_ELEMS + t * 128 * RAG
            return a[off:off + 128 * RAG].rearrange("(p f) -> p f", f=RAG)

        # ============ hop-1 products: node-major + AllGather ============
        # Both supports share one pass over the moving operand.
        def emit_hop1_pair(pid, mov_main, mov_rag, agin_idx, yt_dst):
            """Y1_s[own rows, :] = A_s @ M for s in (0, 1)."""
            # ragged columns, transposed: psum[c(32), n(512)] per support
            ps_t = [psum.tile([RAG, NOWN], F32, name=f"pst{pid}{s}",
                              tag="acc", bufs=8) for s in range(2)]
            for m in range(MT):
                mvr = sb_mov.tile([128, RAG], BF16, name=f"mvr{pid}_{m}",
                                  tag="movr", bufs=8)
                nc.sync.dma_start(mvr[:], mov_rag(m))
                for s in range(2):
                    nc.tensor.matmul(ps_t[s][:], mvr[:], T_tile(s, m),
                                     start=(m == 0), stop=(m == MT - 1))
            for s in range(2):
                rag_ex = sb_sm.tile([RAG, NOWN], BF16, name=f"rgex{pid}{s}",
                                    tag="ragex", bufs=1)
                nc.vector.tensor_copy(rag_ex[:], ps_t[s][:])
                nc.sync.dma_start(yt_dst[s].opt()[MAIN:BC, :], rag_ex[:])
                for t in range(NT):
                    tp = psum.tile([128, RAG], BF16, name=f"rtp{pid}{s}",
                                   tag="acc", bufs=8)
                    nc.tensor.transpose(
                        tp[:], rag_ex[:, t * 128:(t + 1) * 128],
                        identb[0:RAG, 0:RAG])
                    rnm = sb_sm.tile([128, RAG], BF16, name=f"rnm{pid}{s}",
                                     tag="rnm", bufs=2)
                    nc.vector.tensor_copy(rnm[:], tp[:])
                    nc.sync.dma_start(agin_own_rag(agin_idx[s], t), rnm[:])

            # main columns in two 512-wide sweeps; 2 supports x 4 n-tiles
            # of accumulators fill all 8 PSUM banks per sweep
            for hh in range(2):
                ps_m = {}
                for s in range(2):
                    for n in range(NT):
                        ps_m[(s, n)] = psum.tile(
                            [128, 512], F32, name=f"psm{pid}_{hh}{s}{n}",
                            tag="acc", bufs=8)
                for m in range(MT):
                    mv = sb_mov.tile([128, 512], BF16,
                                     name=f"mv{pid}_{hh}_{m}", tag="mov",
                                     bufs=4)
                    nc.sync.dma_start(mv[:], mov_main(m, hh))
                    for s in range(2):
                        for n in range(NT):
                            nc.tensor.matmul(
                                ps_m[(s, n)][:],
                                T_tile(s, m)[:, n * 128:(n + 1) * 128],
                                mv[:], start=(m == 0), stop=(m == MT - 1))
                for s in range(2):
                    exhs = []
                    for n in range(NT):
                        exh = sb_ex.tile([128, 512], BF16,
                                         name=f"ex{pid}{hh}{s}{n}",
                                         tag="ex", bufs=6)
                        nc.vector.tensor_copy(exh[:], ps_m[(s, n)][:])
                        nc.sync.dma_start(
                            agin_own_main_half(agin_idx[s], n, hh), exh[:])
                        exhs.append(exh)
                    # feature-major staging: per bc row-block j, transpose
                    # the 4 n-chunks and write one contiguous row-block
                    for j in range(4):
                        st4 = sb_sm.tile([128, NOWN], BF16,
                                         name=f"st4{pid}", tag="st", bufs=2)
                        for n in range(NT):
                            tp = psum.tile([128, 128], BF16,
                                           name=f"tp{pid}", tag="acc",
                                           bufs=8)
                            nc.tensor.transpose(
                                tp[:], exhs[n][:, j * 128:(j + 1) * 128],
                                identb[:])
                            nc.vector.tensor_copy(
                                st4[:, n * 128:(n + 1) * 128], tp[:])
                        jj = hh * 4 + j
                        nc.sync.dma_start(
                            yt_dst[s].opt()[jj * 128:(jj + 1) * 128, :],
                            st4[:])
            nc.gpsimd.collective_compute(
                "AllGather", mybir.AluOpType.bypass, replica_groups=GROUP,
                ins=[ag_in[agin_idx[0]].opt()],
                outs=[ag_out[agin_idx[0]].opt()])
            nc.gpsimd.collective_compute(
                "AllGather", mybir.AluOpType.bypass, replica_groups=GROUP,
                ins=[ag_in[agin_idx[1]].opt()],
                outs=[ag_out[agin_idx[1]].opt()])

        # ======= hop-2 product: transposed form (feature-major out) =======
        def emit_hop2(pid, s, ag_idx, yt_dst):
            """Y2raw^T[bc, own n] = (A_s @ Y1)^T[bc, n].

            Moving operand = gathered Y1 (ag_out[ag_idx]) loaded as full
            m-rows; its 128-col slices act as lhsT for 8 concurrent
            feature-tile accumulators. The Chebyshev 2x/-x0 combination is
            folded into the dense W host-side."""
            # ragged feature tile (j = JT), its own accumulation
            ps_r = psum.tile([RAG, NOWN], F32, name=f"ph2r{pid}", tag="acc",
                             bufs=8)
            for m in range(MT):
                mvr = sb_mov.tile([128, RAG], BF16, name=f"mvr{pid}_{m}",
                                  tag="movr", bufs=8)
                nc.sync.dma_start(mvr[:], ag_rag_tile(ag_idx, m))
                nc.tensor.matmul(ps_r[:], mvr[:], T_tile(s, m),
                                 start=(m == 0), stop=(m == MT - 1))
            exr = sb_ex.tile([RAG, NOWN], BF16, name=f"h2exr{pid}",
                             tag="ex", bufs=6)
            nc.vector.tensor_copy(exr[:], ps_r[:])
            nc.sync.dma_start(yt_dst.opt()[MAIN:BC, :], exr[:])

            # 8 full feature tiles, m-outer (row loads are contiguous)
            ps = [psum.tile([128, NOWN], F32, name=f"ph2{pid}_{j}",
                            tag="acc", bufs=8) for j in range(JT)]
            for m in range(MT):
                mrow = sb_mov.tile([128, MAIN], BF16, name=f"mr{pid}_{m}",
                                   tag="mov", bufs=4)
                for h in range(2):
                    nc.sync.dma_start(mrow[:, h * 512:(h + 1) * 512],
                                      ag_main_half(ag_idx, m, h))
                for j in range(JT):
                    nc.tensor.matmul(
                        ps[j][:], mrow[:, j * 128:(j + 1) * 128],
                        T_tile(s, m), start=(m == 0), stop=(m == MT - 1))
            for j in range(JT):
                exh = sb_ex.tile([128, NOWN], BF16, name=f"h2ex{pid}_{j}",
                                 tag="ex", bufs=6)
                nc.vector.tensor_copy(exh[:], ps[j][:])
                nc.sync.dma_start(
                    yt_dst.opt()[j * 128:(j + 1) * 128, :], exh[:])

        # ======================= GCN 1 (gate) =======================
        emit_hop1_pair("g1h1", xs_main_half, xs_rag_tile, (0, 1),
                       (yt[0], yt[2]))

        emit_hop2("g1s0h2", 0, 0, yt[1])
        emit_hop2("g1s1h2", 1, 1, yt[3])

        # gate W-stage + candidate build
        for b in range(B):
            xsT_b = sb_sm.tile([C, NOWN], BF16, name="xsTb", tag="xsTb",
                               bufs=2)
            nc.sync.dma_start(xsT_b[:],
                              d["xsT_own"].ap()[b * C:(b + 1) * C, :])
            stT_g = sb_sm.tile([H, NOWN], F32, name="stTg", tag="stg",
                               bufs=2)
            nc.sync.dma_start(stT_g[:], d["stateT"].ap()[b])
            blocks = [xsT_b]
            for j in range(4):
                bt = sb_sm.tile([C, NOWN], BF16, name=f"blk{j}",
                                tag=f"blk{j}", bufs=2)
                nc.sync.dma_start(bt[:], yt[j].opt()[b * C:(b + 1) * C, :])
                blocks.append(bt)
            zr_ps = psum.tile([2 * H, NOWN], F32, name="zrps", tag="acc",
                              bufs=8)
            for j in range(5):
                nc.tensor.matmul(zr_ps[:], wg_t[:, j, :], blocks[j][:],
                                 start=(j == 0), stop=(j == 4))
            zr = sb_sm.tile([2 * H, NOWN], F32, name="zr", tag="zr", bufs=2)
            nc.scalar.activation(zr[:], zr_ps[:], AF.Sigmoid, bias=bg_t[:])
            nc.sync.dma_start(rt_dram.opt()[b], zr[H:2 * H, :])
            # candT_b rows are [z*state(64); x(2)] (host permutes W rows)
            cT = sb_sm.tile([C, NOWN], BF16, name="cT", tag="cT", bufs=2)
            nc.vector.tensor_mul(cT[0:H, :], zr[0:H, :], stT_g[:])
            nc.vector.tensor_copy(cT[H:C, :], xsT_b[H:C, :])
            nc.sync.dma_start(
                candT_dram.opt()[b * C:(b + 1) * C, :], cT[:])
            # cand node-major -> ag_in[2]
            a_main = ag_in[2].opt()[0:MAIN_ELEMS].rearrange(
                "(p f) -> p f", f=MAIN)
            for t in range(NT):
                ps = psum.tile([128, C], BF16, name="ctps", tag="acc",
                               bufs=8)
                nc.tensor.transpose(ps[:], cT[:, t * 128:(t + 1) * 128],
                                    identb[0:C, 0:C])
                ct_nm = sb_sm.tile([128, C], BF16, name="ctnm", tag="ctnm",
                                   bufs=2)
                nc.vector.tensor_copy(ct_nm[:], ps[:])
                lo, hi = b * C, (b + 1) * C
                if hi <= MAIN:
                    nc.sync.dma_start(
                        a_main[t * 128:(t + 1) * 128, lo:hi], ct_nm[:])
                else:
                    cut = MAIN - lo
                    nc.sync.dma_start(
                        a_main[t * 128:(t + 1) * 128, lo:MAIN],
                        ct_nm[:, 0:cut])
                    nc.sync.dma_start(agin_own_rag(2, t),
                                      ct_nm[:, cut:C])
        nc.gpsimd.collective_compute(
            "AllGather", mybir.AluOpType.bypass, replica_groups=GROUP,
            ins=[ag_in[2].opt()], outs=[ag_out[2].opt()])

        # ======================= GCN 2 (update) =======================
        emit_hop1_pair("g2h1",
                       lambda m, h: ag_main_half(2, m, h),
                       lambda m: ag_rag_tile(2, m), (3, 4),
                       (yt2[0], yt2[2]))

        emit_hop2("g2s0h2", 0, 3, yt2[1])
        emit_hop2("g2s1h2", 1, 4, yt2[3])

        # update W-stage + final combine
        for b in range(B):
            cT_b = sb_sm.tile([C, NOWN], BF16, name="cTb", tag="xsTb",
                              bufs=2)
            nc.sync.dma_start(cT_b[:],
                              candT_dram.opt()[b * C:(b + 1) * C, :])
            blocks = [cT_b]
            for j in range(4):
                bt = sb_sm.tile([C, NOWN], BF16, name=f"ublk{j}",
                                tag=f"blk{j}", bufs=2)
                nc.sync.dma_start(bt[:], yt2[j].opt()[b * C:(b + 1) * C, :])
                blocks.append(bt)
            hc_ps = psum.tile([H, NOWN], F32, name="hcps", tag="acc", bufs=8)
            for j in range(5):
                nc.tensor.matmul(hc_ps[:], wu_t[:, j, :], blocks[j][:],
                                 start=(j == 0), stop=(j == 4))
            hc = sb_sm.tile([H, NOWN], F32, name="hc", tag="zr", bufs=2)
            nc.scalar.activation(hc[:], hc_ps[:], AF.Tanh, bias=bu_t[:])

            # out = hc + r * (state - hc)
            stT_u = sb_sm.tile([H, NOWN], F32, name="stTu", tag="stg",
                               bufs=2)
            nc.sync.dma_start(stT_u[:], d["stateT"].ap()[b])
            rT = sb_sm.tile([H, NOWN], F32, name="rT", tag="rT", bufs=2)
            nc.sync.dma_start(rT[:], rt_dram.opt()[b])
            tmp = sb_sm.tile([H, NOWN], F32, name="tmp", tag="tmp", bufs=2)
            nc.vector.tensor_sub(tmp[:], stT_u[:], hc[:])
            nc.vector.tensor_mul(tmp[:], rT[:], tmp[:])
            ot = sb_sm.tile([H, NOWN], F32, name="ot", tag="ot", bufs=2)
            nc.vector.tensor_add(ot[:], hc[:], tmp[:])
            nc.sync.dma_start(d["outT"].ap()[b], ot[:])


def prepare_in_maps(x, state, support0, support1, W_gate, b_gate,
                    W_update, b_update):
    BFNP = mybir.dt.np(BF16)
    xs = np.concatenate([x, state], axis=-1)          # [B, N, C]
    xs_nm = np.ascontiguousarray(
        xs.transpose(1, 0, 2).reshape(N, BC)).astype(BFNP)
    # feature-major input for W / elementwise uses [state(64); x(2)] rows
    sx_nm = np.ascontiguousarray(
        np.concatenate([state, x], axis=-1)
        .transpose(1, 0, 2).reshape(N, BC)).astype(np.float32)
    perm = np.r_[DIN:C, 0:DIN]                 # [x, state] -> [state, x]

    # fold the Chebyshev combination x2 = 2*A@x1 - x0 into W:
    # W0 -= (W2 + W4); W2 *= 2; W4 *= 2  (per 66-row block)
    def fold(W):
        Wf = np.ascontiguousarray(W, dtype=np.float32).copy()
        Wf[0:C] -= Wf[2 * C:3 * C] + Wf[4 * C:5 * C]
        Wf[2 * C:3 * C] *= 2.0
        Wf[4 * C:5 * C] *= 2.0
        return Wf

    Wg_dev = fold(W_gate)
    Wg_dev[0:C] = Wg_dev[0:C][perm]            # only the X-block reads xsT
    Wu_dev = fold(W_update)
    for j in range(5):                         # all of cand's blocks permute
        Wu_dev[j * C:(j + 1) * C] = Wu_dev[j * C:(j + 1) * C][perm]
    Wg_dev = Wg_dev.astype(BFNP)
    Wu_dev = Wu_dev.astype(BFNP)

    xs_main = np.ascontiguousarray(xs_nm[:, :MAIN])
    xs_rag = np.ascontiguousarray(xs_nm[:, MAIN:]).reshape(MT, 128, RAG)
    bg = np.ascontiguousarray(b_gate, dtype=np.float32).reshape(2 * H, 1)
    bu = np.ascontiguousarray(b_update, dtype=np.float32).reshape(H, 1)
    s0b = np.asarray(support0, dtype=np.float32).astype(BFNP)
    s1b = np.asarray(support1, dtype=np.float32).astype(BFNP)
    state_f = np.asarray(state, dtype=np.float32)

    in_maps = []
    for r in range(NCORES):
        n0 = r * NOWN
        in_maps.append({
            "Ts": np.ascontiguousarray(
                np.stack([s0b[n0:n0 + NOWN, :].T,
                          s1b[n0:n0 + NOWN, :].T])),
            "xs_main": xs_main,
            "xs_rag": xs_rag,
            "xsT_own": np.ascontiguousarray(
                sx_nm[n0:n0 + NOWN].T).astype(BFNP),
            "stateT": np.ascontiguousarray(
                state_f[:, n0:n0 + NOWN, :].transpose(0, 2, 1)),
            "Wg": Wg_dev, "bg": bg, "Wu": Wu_dev, "bu": bu,
        })
    return in_maps


def assemble_output(results):
    out = np.empty((B, N, H), dtype=np.float32)
    for r in range(NCORES):
        n0 = r * NOWN
        out[:, n0:n0 + NOWN, :] = results[r]["outT"].transpose(0, 2, 1)
    return out


def get_nc():
    if "nc" not in _NC_CACHE:
        _NC_CACHE["nc"] = build_nc()
    return _NC_CACHE["nc"]


def kernel(x, state, support0, support1, W_gate, b_gate, W_update, b_update):
    nc = get_nc()
    in_maps = prepare_in_maps(x, state, support0, support1,
                              W_gate, b_gate, W_update, b_update)
    prev = os.environ.get("BASS_NEVER_TRACE")
    os.environ["BASS_NEVER_TRACE"] = "1"
    try:
        res = run_bass_kernel_spmd(nc, in_maps, list(range(NCORES)),
                                   trace=False)
    finally:
        if prev is None:
            os.environ.pop("BASS_NEVER_TRACE", None)
        else:
            os.environ["BASS_NEVER_TRACE"] = prev
    return assemble_output(res.results)


# revision 10
# speedup vs baseline: 1.6947x; 1.3933x over previous
"""DCGRU cell (nn_DCGRUCell) Trainium2 Bass kernel, 8 NeuronCores.

Sharding: node dimension N=4096 split 8 ways (512 rows/core); supports are
fed host-transposed (T = A^T), held resident in SBUF as bf16. Hop-1
diffusion products are computed node-major and AllGathered in two
column-chunks per GCN (A: bc cols 0-511; B: cols 512-1023 + 32 ragged) so
the gathers overlap hop-1/hop-2 compute. Hop-2 products are computed
directly in transposed (feature-major) form. All matmul operands are bf16
(PSUM fp32); the Chebyshev combination x2 = 2*A@x1 - x0 is folded into the
dense W matrices host-side. The dense W stage processes batches in pairs
with DMA issue split across the two HWDGE rings (sync=loads,
scalar=stores).

kernel(**inputs) takes the FULL inputs from reference.setup_inputs() and
returns the FULL [16, 4096, 64] float32 output.
"""
import os
import numpy as np

import concourse.bass as bass
import concourse.mybir as mybir
import concourse.tile as tile
from concourse import bacc
from concourse.bass_utils import run_bass_kernel_spmd

F32 = mybir.dt.float32
BF16 = mybir.dt.bfloat16
AF = mybir.ActivationFunctionType

NCORES = 8
B, N, H, DIN = 16, 4096, 64, 2
C = DIN + H                 # 66 features per batch into each GCN
BC = B * C                  # 1056
NOWN = N // NCORES          # 512 rows per core
NT = NOWN // 128            # 4 n-tiles per core
MT = N // 128               # 32 m-tiles (contraction)
CA = 512                    # chunk A: bc columns 0:512
CB = 544                    # chunk B: bc columns 512:1024 + 32 ragged
MAIN = 1024
RAG = BC - MAIN             # 32 ragged columns
GROUP = [list(range(NCORES))]

_NC_CACHE = {}


def build_nc():
    nc = bacc.Bacc("TRN2", target_bir_lowering=False, debug=False,
                   num_devices=NCORES)

    d = {}
    d["Ts"] = nc.dram_tensor("Ts", [2, N, NOWN], BF16, kind="ExternalInput")
    d["xs_main"] = nc.dram_tensor("xs_main", [N, MAIN], BF16,
                                  kind="ExternalInput")
    d["xs_rag"] = nc.dram_tensor("xs_rag", [MT, 128, RAG], BF16,
                                 kind="ExternalInput")
    d["xsT_own"] = nc.dram_tensor("xsT_own", [BC, NOWN], BF16,
                                  kind="ExternalInput")
    d["stateT"] = nc.dram_tensor("stateT", [B, H, NOWN], F32,
                                 kind="ExternalInput")
    d["Wg"] = nc.dram_tensor("Wg", [5 * C, 2 * H], BF16, kind="ExternalInput")
    d["bg"] = nc.dram_tensor("bg", [2 * H, 1], F32, kind="ExternalInput")
    d["Wu"] = nc.dram_tensor("Wu", [5 * C, H], BF16, kind="ExternalInput")
    d["bu"] = nc.dram_tensor("bu", [H, 1], F32, kind="ExternalInput")
    d["outT"] = nc.dram_tensor("outT", [B, H, NOWN], F32,
                               kind="ExternalOutput")

    with tile.TileContext(nc) as tc:
        _emit(nc, tc, d)
    nc.compile()
    return nc


def _emit(nc, tc, d):
    import contextlib
    stack = contextlib.ExitStack()
    with stack:
        const = stack.enter_context(tc.tile_pool(name="const", bufs=1))
        sb_ex = stack.enter_context(tc.tile_pool(name="ex", bufs=1))
        sb_mov = stack.enter_context(tc.tile_pool(name="mov", bufs=1))
        sb_sm = stack.enter_context(tc.tile_pool(name="small", bufs=1))
        dram = stack.enter_context(
            tc.tile_pool(name="dram", bufs=1, space="DRAM"))
        psum = stack.enter_context(
            tc.tile_pool(name="psum", bufs=1, space="PSUM"))

        # ---- resident support tiles (loaded staggered in first sweep) ----
        CH = 4
        NCH = MT // CH
        Tch = {}
        for s in range(2):
            for k in range(NCH):
                Tch[(s, k)] = const.tile([128, CH, 512], BF16,
                                         name=f"T{s}_{k}")

        def load_Tch(k):
            for s in range(2):
                ts = d["Ts"].ap()[s].rearrange("(t p) n -> p t n", p=128)
                nc.sync.dma_start(Tch[(s, k)][:],
                                  ts[:, k * CH:(k + 1) * CH, :])

        load_Tch(0)
        load_Tch(1)

        def T_tile(s, m):
            return Tch[(s, m // CH)][:, m % CH, :]

        ident = const.tile([128, 128], F32)
        nc.gpsimd.memset(ident[:], 0.0)
        nc.gpsimd.affine_select(
            out=ident[:], in_=ident[:],
            compare_op=mybir.AluOpType.not_equal, fill=1.0, base=0,
            pattern=[[-1, 128]], channel_multiplier=1)
        identb = const.tile([128, 128], BF16)
        nc.vector.tensor_copy(identb[:], ident[:])

        # dense-W constants on the scalar (act) HWDGE ring
        wg_t = const.tile([C, 5, 2 * H], BF16)
        wu_t = const.tile([C, 5, H], BF16)
        for j in range(5):
            nc.scalar.dma_start(wg_t[:, j, :],
                                d["Wg"].ap()[j * C:(j + 1) * C, :])
            nc.scalar.dma_start(wu_t[:, j, :],
                                d["Wu"].ap()[j * C:(j + 1) * C, :])
        bg_t = const.tile([2 * H, 1], F32)
        nc.scalar.dma_start(bg_t[:], d["bg"].ap())
        bu_t = const.tile([H, 1], F32)
        nc.scalar.dma_start(bu_t[:], d["bu"].ap())

        # ---- DRAM staging ----
        # Per GCN g: chunk A = [s, t, 128, CA] node-major y1 cols 0:512,
        # chunk B = [s, t, 128, CB] cols 512:1024 + ragged 32.
        agA = [dram.tile([2 * NT * 128 * CA], BF16, name=f"agA{g}")
               for g in range(2)]
        agB = [dram.tile([2 * NT * 128 * CB], BF16, name=f"agB{g}")
               for g in range(2)]
        agAo = [dram.tile([NCORES * 2 * NT * 128 * CA], BF16,
                          name=f"agAo{g}", addr_space="Shared")
                for g in range(2)]
        agBo = [dram.tile([NCORES * 2 * NT * 128 * CB], BF16,
                          name=f"agBo{g}", addr_space="Shared")
                for g in range(2)]
        candA = dram.tile([NT * 128 * CA], BF16, name="candA")
        candB = dram.tile([NT * 128 * CB], BF16, name="candB")
        candAo = dram.tile([NCORES * NT * 128 * CA], BF16, name="candAo",
                           addr_space="Shared")
        candBo = dram.tile([NCORES * NT * 128 * CB], BF16, name="candBo",
                           addr_space="Shared")
        yt = [dram.tile([BC, NOWN], BF16, name=f"yt{i}") for i in range(4)]
        yt2 = [dram.tile([BC, NOWN], BF16, name=f"yt2_{i}") for i in range(4)]
        candT_dram = dram.tile([BC, NOWN], BF16)
        rt_dram = dram.tile([B, H, NOWN], F32)

        def agA_own(g, s, t):
            o = ((s * NT + t) * 128) * CA
            return agA[g].opt()[o:o + 128 * CA].rearrange(
                "(p f) -> p f", f=CA)

        def agB_own(g, s, t):
            o = ((s * NT + t) * 128) * CB
            return agB[g].opt()[o:o + 128 * CB].rearrange(
                "(p f) -> p f", f=CB)

        def outA_blk(g, s, m):
            q, t = divmod(m, NT)
            o = (((q * 2 + s) * NT + t) * 128) * CA
            return agAo[g].opt()[o:o + 128 * CA].rearrange(
                "(p f) -> p f", f=CA)

        def outB_blk(g, s, m):
            q, t = divmod(m, NT)
            o = (((q * 2 + s) * NT + t) * 128) * CB
            return agBo[g].opt()[o:o + 128 * CB].rearrange(
                "(p f) -> p f", f=CB)

        def candA_own(t):
            o = t * 128 * CA
            return candA.opt()[o:o + 128 * CA].rearrange(
                "(p f) -> p f", f=CA)

        def candB_own(t):
            o = t * 128 * CB
            return candB.opt()[o:o + 128 * CB].rearrange(
                "(p f) -> p f", f=CB)

        def candAo_blk(m):
            q, t = divmod(m, NT)
            o = (q * NT + t) * 128 * CA
            return candAo.opt()[o:o + 128 * CA].rearrange(
                "(p f) -> p f", f=CA)

        def candBo_blk(m):
            q, t = divmod(m, NT)
            o = (q * NT + t) * 128 * CB
            return candBo.opt()[o:o + 128 * CB].rearrange(
                "(p f) -> p f", f=CB)

        def allgather(src, dst):
            nc.gpsimd.collective_compute(
                "AllGather", mybir.AluOpType.bypass, replica_groups=GROUP,
                ins=[src.opt()], outs=[dst.opt()])

        # ============ hop-1: node-major chunks + overlapped AG ============
        def emit_hop1_pair(pid, g, mov_main, mov_rag, yt_dst, stagger_T):
            """Y1_s[own rows, :] = A_s @ M for s in (0, 1); AG chunk A
            issued between the two main sweeps, chunk B after the ragged
            pass. Feature-major yt transposes are deferred (returned as a
            closure) so they land on the PE during the AG-B transfer."""
            kept = {}
            for hh in range(2):
                ps_m = {}
                for s in range(2):
                    for n in range(NT):
                        ps_m[(s, n)] = psum.tile(
                            [128, 512], F32, name=f"psm{pid}_{hh}{s}{n}",
                            tag="acc", bufs=8)
                for m in range(MT):
                    if stagger_T and hh == 0 and m % CH == 0:
                        k = m // CH + 2
                        if k < NCH:
                            load_Tch(k)
                    mv = sb_mov.tile([128, 512], BF16,
                                     name=f"mv{pid}_{hh}_{m}", tag="mov",
                                     bufs=6)
                    nc.sync.dma_start(mv[:], mov_main(m, hh))
                    for s in range(2):
                        for n in range(NT):
                            nc.tensor.matmul(
                                ps_m[(s, n)][:],
                                T_tile(s, m)[:, n * 128:(n + 1) * 128],
                                mv[:], start=(m == 0), stop=(m == MT - 1))
                for s in range(2):
                    exhs = []
                    for n in range(NT):
                        exh = sb_ex.tile([128, 512], BF16,
                                         name=f"ex{pid}{hh}{s}{n}",
                                         tag="ex", bufs=20)
                        nc.vector.tensor_copy(exh[:], ps_m[(s, n)][:])
                        dst = (agA_own(g, s, n) if hh == 0
                               else agB_own(g, s, n)[:, 0:512])
                        nc.scalar.dma_start(dst, exh[:])
                        exhs.append(exh)
                    kept[(hh, s)] = exhs
                if hh == 0:
                    allgather(agA[g], agAo[g])

            # ragged pass (node-major staging feeds chunk B)
            ps_t = [psum.tile([RAG, NOWN], F32, name=f"pst{pid}{s}",
                              tag="acc", bufs=8) for s in range(2)]
            for m in range(MT):
                mvr = sb_mov.tile([128, RAG], BF16, name=f"mvr{pid}_{m}",
                                  tag="movr", bufs=8)
                nc.sync.dma_start(mvr[:], mov_rag(m))
                for s in range(2):
                    nc.tensor.matmul(ps_t[s][:], mvr[:], T_tile(s, m),
                                     start=(m == 0), stop=(m == MT - 1))
            rag_exs = {}
            for s in range(2):
                rag_ex = sb_ex.tile([RAG, NOWN], BF16, name=f"rgex{pid}{s}",
                                    tag="ragex", bufs=2)
                nc.vector.tensor_copy(rag_ex[:], ps_t[s][:])
                nc.scalar.dma_start(yt_dst[s].opt()[MAIN:BC, :], rag_ex[:])
                rag_exs[s] = rag_ex
                for t in range(NT):
                    tp = psum.tile([128, RAG], BF16, name=f"rtp{pid}{s}",
                                   tag="acc", bufs=8)
                    nc.tensor.transpose(
                        tp[:], rag_ex[:, t * 128:(t + 1) * 128],
                        identb[0:RAG, 0:RAG])
                    rnm = sb_sm.tile([128, RAG], BF16, name=f"rnm{pid}{s}",
                                     tag="rnm", bufs=2)
                    nc.vector.tensor_copy(rnm[:], tp[:])
                    nc.scalar.dma_start(agB_own(g, s, t)[:, 512:544],
                                        rnm[:])
            allgather(agB[g], agBo[g])

            def deferred_yt():
                # feature-major staging: per bc row-block, transpose the
                # 4 n-chunks and write one contiguous row-block
                for s in range(2):
                    for hh in range(2):
                        for j in range(4):
                            st4 = sb_sm.tile([128, NOWN], BF16,
                                             name=f"st4{pid}", tag="st",
                                             bufs=2)
                            for n in range(NT):
                                tp = psum.tile([128, 128], BF16,
                                               name=f"tp{pid}", tag="acc",
                                               bufs=8)
                                nc.tensor.transpose(
                                    tp[:],
                                    kept[(hh, s)][n][:,
                                                     j * 128:(j + 1) * 128],
                                    identb[:])
                                nc.vector.tensor_copy(
                                    st4[:, n * 128:(n + 1) * 128], tp[:])
                            jj = hh * 4 + j
                            nc.scalar.dma_start(
                                yt_dst[s].opt()[jj * 128:(jj + 1) * 128, :],
                                st4[:])
            return deferred_yt

        # ======= hop-2 product: transposed form (feature-major out) =======
        def emit_hop2_pass(pid, s, g, part, yt_dst):
            """Y2raw^T[bc, own n] = (A_s @ Y1)^T for one column chunk."""
            if part == 0:
                ps = [psum.tile([128, NOWN], F32, name=f"ph2{pid}_{j}",
                                tag="acc", bufs=8) for j in range(4)]
                for m in range(MT):
                    mr = sb_mov.tile([128, CA], BF16, name=f"mrA{pid}_{m}",
                                     tag="mov", bufs=6)
                    nc.sync.dma_start(mr[:], outA_blk(g, s, m))
                    for j in range(4):
                        nc.tensor.matmul(
                            ps[j][:], mr[:, j * 128:(j + 1) * 128],
                            T_tile(s, m), start=(m == 0),
                            stop=(m == MT - 1))
                for j in range(4):
                    exh = sb_ex.tile([128, NOWN], BF16,
                                     name=f"h2ex{pid}_{j}", tag="ex",
                                     bufs=20)
                    nc.vector.tensor_copy(exh[:], ps[j][:])
                    nc.scalar.dma_start(
                        yt_dst.opt()[j * 128:(j + 1) * 128, :], exh[:])
            else:
                ps = [psum.tile([128, NOWN], F32, name=f"ph2{pid}_{j}",
                                tag="acc", bufs=8) for j in range(4)]
                ps_r = psum.tile([RAG, NOWN], F32, name=f"ph2r{pid}",
                                 tag="acc", bufs=8)
                for m in range(MT):
                    mr = sb_mov.tile([128, CB], BF16, name=f"mrB{pid}_{m}",
                                     tag="movB", bufs=6)
                    nc.sync.dma_start(mr[:], outB_blk(g, s, m))
                    for j in range(4):
                        nc.tensor.matmul(
                            ps[j][:], mr[:, j * 128:(j + 1) * 128],
                            T_tile(s, m), start=(m == 0),
                            stop=(m == MT - 1))
                    nc.tensor.matmul(ps_r[:], mr[:, 512:544], T_tile(s, m),
                                     start=(m == 0), stop=(m == MT - 1))
                for j in range(4):
                    exh = sb_ex.tile([128, NOWN], BF16,
                                     name=f"h2ex{pid}_{j}", tag="ex",
                                     bufs=20)
                    nc.vector.tensor_copy(exh[:], ps[j][:])
                    nc.scalar.dma_start(
                        yt_dst.opt()[(4 + j) * 128:(5 + j) * 128, :],
                        exh[:])
                exr = sb_ex.tile([RAG, NOWN], BF16, name=f"h2exr{pid}",
                                 tag="ragex", bufs=2)
                nc.vector.tensor_copy(exr[:], ps_r[:])
                nc.scalar.dma_start(yt_dst.opt()[MAIN:BC, :], exr[:])

        # ======================= GCN 1 (gate) =======================
        def g1_main(m, hh):
            return d["xs_main"].ap()[m * 128:(m + 1) * 128,
                                     hh * 512:(hh + 1) * 512]

        def g1_rag(m):
            return d["xs_rag"].ap()[m]

        dyt1 = emit_hop1_pair("g1h1", 0, g1_main, g1_rag,
                              (yt[0], yt[2]), True)
        dyt1()
        emit_hop2_pass("g1s0h2A", 0, 0, 0, yt[1])
        emit_hop2_pass("g1s1h2A", 1, 0, 0, yt[3])
        emit_hop2_pass("g1s0h2B", 0, 0, 1, yt[1])
        emit_hop2_pass("g1s1h2B", 1, 0, 1, yt[3])

        # gate W-stage + candidate build, batches in pairs
        for pb in range(B // 2):
            b0 = 2 * pb
            xsT2 = sb_sm.tile([C, 2, NOWN], BF16, name="xsTb", tag="xsTb",
                              bufs=2)
            nc.sync.dma_start(
                xsT2[:],
                d["xsT_own"].ap()[b0 * C:(b0 + 2) * C, :]
                .rearrange("(b c) n -> c b n", b=2))
            stT2 = sb_sm.tile([H, 2, NOWN], F32, name="stTg", tag="stg",
                              bufs=2)
            nc.sync.dma_start(
                stT2[:],
                d["stateT"].ap()[b0:b0 + 2].rearrange("b h n -> h b n"))
            blocks = [xsT2]
            for j in range(4):
                bt = sb_sm.tile([C, 2, NOWN], BF16, name=f"blk{j}",
                                tag=f"blk{j}", bufs=2)
                nc.sync.dma_start(
                    bt[:],
                    yt[j].opt()[b0 * C:(b0 + 2) * C, :]
                    .rearrange("(b c) n -> c b n", b=2))
                blocks.append(bt)
            zr_ps = [psum.tile([2 * H, NOWN], F32, name=f"zrps{b2}",
                               tag="acc", bufs=8) for b2 in range(2)]
            for j in range(5):
                for b2 in range(2):
                    nc.tensor.matmul(zr_ps[b2][:], wg_t[:, j, :],
                                     blocks[j][:, b2, :],
                                     start=(j == 0), stop=(j == 4))
            zr2 = sb_sm.tile([2 * H, 2, NOWN], F32, name="zr", tag="zr",
                             bufs=2)
            for b2 in range(2):
                nc.scalar.activation(zr2[:, b2, :], zr_ps[b2][:],
                                     AF.Sigmoid, bias=bg_t[:])
            nc.scalar.dma_start(
                rt_dram.opt()[b0:b0 + 2].rearrange("b (h n) -> h b n", h=H),
                zr2[H:2 * H, :, :])
            # candT rows are [z*state(64); x(2)] (host permutes W rows)
            cT2 = sb_sm.tile([C, 2, NOWN], BF16, name="cT", tag="cT",
                             bufs=2)
            nc.vector.tensor_mul(cT2[0:H, :, :], zr2[0:H, :, :], stT2[:])
            nc.vector.tensor_copy(cT2[H:C, :, :], xsT2[H:C, :, :])
            nc.scalar.dma_start(
                candT_dram.opt()[b0 * C:(b0 + 2) * C, :]
                .rearrange("(b c) n -> c b n", b=2), cT2[:])
            # cand node-major -> candA/candB chunks
            c0 = b0 * C
            for t in range(NT):
                ct2 = sb_sm.tile([128, 2, C], BF16, name="ctnm", tag="ctnm",
                                 bufs=2)
                for b2 in range(2):
                    tp = psum.tile([128, C], BF16, name="ctps", tag="acc",
                                   bufs=8)
                    nc.tensor.transpose(
                        tp[:], cT2[:, b2, t * 128:(t + 1) * 128],
                        identb[0:C, 0:C])
                    nc.vector.tensor_copy(ct2[:, b2, :], tp[:])
                flat = ct2[:].rearrange("p b c -> p (b c)")
                # split the 132 columns across chunk boundaries
                segs = []
                lo = c0
                hi = c0 + 2 * C
                if lo < CA:
                    e = min(hi, CA)
                    segs.append((candA_own(t)[:, lo:e], 0, e - lo))
                if hi > CA and lo < MAIN:
                    s0 = max(lo, CA)
                    e = min(hi, MAIN)
                    segs.append((candB_own(t)[:, s0 - CA:e - CA],
                                 s0 - lo, e - s0))
                if hi > MAIN:
                    s0 = max(lo, MAIN)
                    segs.append((candB_own(t)[:, 512 + s0 - MAIN:
                                              512 + hi - MAIN],
                                 s0 - lo, hi - s0))
                for dst, off, w in segs:
                    nc.scalar.dma_start(dst, flat[:, off:off + w])
            if pb == 3:
                allgather(candA, candAo)
        allgather(candB, candBo)

        # ======================= GCN 2 (update) =======================
        def g2_main(m, hh):
            if hh == 0:
                return candAo_blk(m)
            return candBo_blk(m)[:, 0:512]

        def g2_rag(m):
            return candBo_blk(m)[:, 512:544]

        dyt2 = emit_hop1_pair("g2h1", 1, g2_main, g2_rag,
                              (yt2[0], yt2[2]), False)
        dyt2()
        emit_hop2_pass("g2s0h2A", 0, 1, 0, yt2[1])
        emit_hop2_pass("g2s1h2A", 1, 1, 0, yt2[3])
        emit_hop2_pass("g2s0h2B", 0, 1, 1, yt2[1])
        emit_hop2_pass("g2s1h2B", 1, 1, 1, yt2[3])

        # update W-stage + final combine, batches in pairs
        for pb in range(B // 2):
            b0 = 2 * pb
            cT2b = sb_sm.tile([C, 2, NOWN], BF16, name="cTb", tag="xsTb",
                              bufs=2)
            nc.sync.dma_start(
                cT2b[:],
                candT_dram.opt()[b0 * C:(b0 + 2) * C, :]
                .rearrange("(b c) n -> c b n", b=2))
            blocks = [cT2b]
            for j in range(4):
                bt = sb_sm.tile([C, 2, NOWN], BF16, name=f"ublk{j}",
                                tag=f"blk{j}", bufs=2)
                nc.sync.dma_start(
                    bt[:],
                    yt2[j].opt()[b0 * C:(b0 + 2) * C, :]
                    .rearrange("(b c) n -> c b n", b=2))
                blocks.append(bt)
            hc_ps = [psum.tile([H, NOWN], F32, name=f"hcps{b2}", tag="acc",
                               bufs=8) for b2 in range(2)]
            for j in range(5):
                for b2 in range(2):
                    nc.tensor.matmul(hc_ps[b2][:], wu_t[:, j, :],
                                     blocks[j][:, b2, :],
                                     start=(j == 0), stop=(j == 4))
            hc2 = sb_sm.tile([H, 2, NOWN], F32, name="hc", tag="zr",
                             bufs=2)
            for b2 in range(2):
                nc.scalar.activation(hc2[:, b2, :], hc_ps[b2][:], AF.Tanh,
                                     bias=bu_t[:])

            # out = hc + r * (state - hc)
            stT2 = sb_sm.tile([H, 2, NOWN], F32, name="stTu", tag="stg",
                              bufs=2)
            nc.sync.dma_start(
                stT2[:],
                d["stateT"].ap()[b0:b0 + 2].rearrange("b h n -> h b n"))
            rT2 = sb_sm.tile([H, 2, NOWN], F32, name="rT", tag="rT",
                             bufs=2)
            nc.sync.dma_start(
                rT2[:],
                rt_dram.opt()[b0:b0 + 2].rearrange("b (h n) -> h b n",
                                                   h=H))
            tmp2 = sb_sm.tile([H, 2, NOWN], F32, name="tmp", tag="tmp",
                              bufs=2)
            nc.vector.tensor_sub(tmp2[:], stT2[:], hc2[:])
            nc.vector.tensor_mul(tmp2[:], rT2[:], tmp2[:])
            ot2 = sb_sm.tile([H, 2, NOWN], F32, name="ot", tag="ot",
                             bufs=2)
            nc.vector.tensor_add(ot2[:], hc2[:], tmp2[:])
            nc.scalar.dma_start(
                d["outT"].ap()[b0:b0 + 2].rearrange("b h n -> h b n"),
                ot2[:])


def prepare_in_maps(x, state, support0, support1, W_gate, b_gate,
                    W_update, b_update):
    BFNP = mybir.dt.np(BF16)
    xs = np.concatenate([x, state], axis=-1)          # [B, N, C]
    xs_nm = np.ascontiguousarray(
        xs.transpose(1, 0, 2).reshape(N, BC)).astype(BFNP)
    # feature-major input for W / elementwise uses [state(64); x(2)] rows
    sx_nm = np.ascontiguousarray(
        np.concatenate([state, x], axis=-1)
        .transpose(1, 0, 2).reshape(N, BC)).astype(np.float32)
    perm = np.r_[DIN:C, 0:DIN]                 # [x, state] -> [state, x]

    # fold the Chebyshev combination x2 = 2*A@x1 - x0 into W:
    # W0 -= (W2 + W4); W2 *= 2; W4 *= 2  (per 66-row block)
    def fold(W):
        Wf = np.ascontiguousarray(W, dtype=np.float32).copy()
        Wf[0:C] -= Wf[2 * C:3 * C] + Wf[4 * C:5 * C]
        Wf[2 * C:3 * C] *= 2.0
        Wf[4 * C:5 * C] *= 2.0
        return Wf

    Wg_dev = fold(W_gate)
    Wg_dev[0:C] = Wg_dev[0:C][perm]            # only the X-block reads xsT
    Wu_dev = fold(W_update)
    for j in range(5):                         # all of cand's blocks permute
        Wu_dev[j * C:(j + 1) * C] = Wu_dev[j * C:(j + 1) * C][perm]
    Wg_dev = Wg_dev.astype(BFNP)
    Wu_dev = Wu_dev.astype(BFNP)

    xs_main = np.ascontiguousarray(xs_nm[:, :MAIN])
    xs_rag = np.ascontiguousarray(xs_nm[:, MAIN:]).reshape(MT, 128, RAG)
    bg = np.ascontiguousarray(b_gate, dtype=np.float32).reshape(2 * H, 1)
    bu = np.ascontiguousarray(b_update, dtype=np.float32).reshape(H, 1)
    s0b = np.asarray(support0, dtype=np.float32).astype(BFNP)
    s1b = np.asarray(support1, dtype=np.float32).astype(BFNP)
    state_f = np.asarray(state, dtype=np.float32)

    in_maps = []
    for r in range(NCORES):
        n0 = r * NOWN
        in_maps.append({
            "Ts": np.ascontiguousarray(
                np.stack([s0b[n0:n0 + NOWN, :].T,
                          s1b[n0:n0 + NOWN, :].T])),
            "xs_main": xs_main,
            "xs_rag": xs_rag,
            "xsT_own": np.ascontiguousarray(
                sx_nm[n0:n0 + NOWN].T).astype(BFNP),
            "stateT": np.ascontiguousarray(
                state_f[:, n0:n0 + NOWN, :].transpose(0, 2, 1)),
            "Wg": Wg_dev, "bg": bg, "Wu": Wu_dev, "bu": bu,
        })
    return in_maps


def assemble_output(results):
    out = np.empty((B, N, H), dtype=np.float32)
    for r in range(NCORES):
        n0 = r * NOWN
        out[:, n0:n0 + NOWN, :] = results[r]["outT"].transpose(0, 2, 1)
    return out


def get_nc():
    if "nc" not in _NC_CACHE:
        _NC_CACHE["nc"] = build_nc()
    return _NC_CACHE["nc"]


def kernel(x, state, support0, support1, W_gate, b_gate, W_update, b_update):
    nc = get_nc()
    in_maps = prepare_in_maps(x, state, support0, support1,
                              W_gate, b_gate, W_update, b_update)
    prev = os.environ.get("BASS_NEVER_TRACE")
    os.environ["BASS_NEVER_TRACE"] = "1"
    try:
        res = run_bass_kernel_spmd(nc, in_maps, list(range(NCORES)),
                                   trace=False)
    finally:
        if prev is None:
            os.environ.pop("BASS_NEVER_TRACE", None)
        else:
            os.environ["BASS_NEVER_TRACE"] = prev
    return assemble_output(res.results)


# revision 12
# speedup vs baseline: 1.7186x; 1.0141x over previous
"""DCGRU cell (nn_DCGRUCell) Trainium2 Bass kernel, 8 NeuronCores.

Sharding: node dimension N=4096 split 8 ways (512 rows/core); supports are
fed host-transposed (T = A^T), held resident in SBUF as bf16. Hop-1
diffusion products are computed node-major and AllGathered in two
column-chunks per GCN (A: bc cols 0-511; B: cols 512-1023 + 32 ragged) so
the gathers overlap hop-1/hop-2 compute. Hop-2 products are computed
directly in transposed (feature-major) form. All matmul operands are bf16
(PSUM fp32); the Chebyshev combination x2 = 2*A@x1 - x0 is folded into the
dense W matrices host-side. Moving operands are loaded 4 m-tiles per DMA;
the dense W stage processes batches 4 at a time with the candidate
transposes software-pipelined one iteration behind the matmuls. DMA issue
is split across the two HWDGE rings (sync=loads, scalar=stores).

kernel(**inputs) takes the FULL inputs from reference.setup_inputs() and
returns the FULL [16, 4096, 64] float32 output.
"""
import os
import numpy as np

import concourse.bass as bass
import concourse.mybir as mybir
import concourse.tile as tile
from concourse import bacc
from concourse.bass_utils import run_bass_kernel_spmd

F32 = mybir.dt.float32
BF16 = mybir.dt.bfloat16
AF = mybir.ActivationFunctionType

NCORES = 8
B, N, H, DIN = 16, 4096, 64, 2
C = DIN + H                 # 66 features per batch into each GCN
BC = B * C                  # 1056
NOWN = N // NCORES          # 512 rows per core
NT = NOWN // 128            # 4 n-tiles per core
MT = N // 128               # 32 m-tiles (contraction)
NQ = MT // NT               # 8 rank-blocks of 4 m-tiles
CA = 512                    # chunk A: bc columns 0:512
CB = 544                    # chunk B: bc columns 512:1024 + 32 ragged
MAIN = 1024
RAG = BC - MAIN             # 32 ragged columns
NB = 4                      # batches per W-stage iteration
GROUP = [list(range(NCORES))]

_NC_CACHE = {}


def build_nc():
    nc = bacc.Bacc("TRN2", target_bir_lowering=False, debug=False,
                   num_devices=NCORES)

    d = {}
    d["Ts"] = nc.dram_tensor("Ts", [2, N, NOWN], BF16, kind="ExternalInput")
    d["xs_main"] = nc.dram_tensor("xs_main", [N, MAIN], BF16,
                                  kind="ExternalInput")
    d["xs_rag"] = nc.dram_tensor("xs_rag", [MT, 128, RAG], BF16,
                                 kind="ExternalInput")
    d["xsT_own"] = nc.dram_tensor("xsT_own", [BC, NOWN], BF16,
                                  kind="ExternalInput")
    d["Wg"] = nc.dram_tensor("Wg", [5 * C, 2 * H], BF16, kind="ExternalInput")
    d["bg"] = nc.dram_tensor("bg", [2 * H, 1], F32, kind="ExternalInput")
    d["Wu"] = nc.dram_tensor("Wu", [5 * C, H], BF16, kind="ExternalInput")
    d["bu"] = nc.dram_tensor("bu", [H, 1], F32, kind="ExternalInput")
    d["outT"] = nc.dram_tensor("outT", [B, H, NOWN], F32,
                               kind="ExternalOutput")

    with tile.TileContext(nc) as tc:
        _emit(nc, tc, d)
    nc.compile()
    return nc


def _emit(nc, tc, d):
    import contextlib
    stack = contextlib.ExitStack()
    with stack:
        const = stack.enter_context(tc.tile_pool(name="const", bufs=1))
        sb_ex = stack.enter_context(tc.tile_pool(name="ex", bufs=1))
        sb_mov = stack.enter_context(tc.tile_pool(name="mov", bufs=1))
        sb_sm = stack.enter_context(tc.tile_pool(name="small", bufs=1))
        dram = stack.enter_context(
            tc.tile_pool(name="dram", bufs=1, space="DRAM"))
        psum = stack.enter_context(
            tc.tile_pool(name="psum", bufs=1, space="PSUM"))

        # ---- resident support tiles (loaded staggered in first sweep) ----
        Tch = {}
        for s in range(2):
            for k in range(NQ):
                Tch[(s, k)] = const.tile([128, NT, 512], BF16,
                                         name=f"T{s}_{k}")

        def load_Tch(k):
            for s in range(2):
                ts = d["Ts"].ap()[s].rearrange("(t p) n -> p t n", p=128)
                nc.sync.dma_start(Tch[(s, k)][:],
                                  ts[:, k * NT:(k + 1) * NT, :])

        load_Tch(0)
        load_Tch(1)

        def T_tile(s, m):
            return Tch[(s, m // NT)][:, m % NT, :]

        ident = const.tile([128, 128], F32)
        nc.gpsimd.memset(ident[:], 0.0)
        nc.gpsimd.affine_select(
            out=ident[:], in_=ident[:],
            compare_op=mybir.AluOpType.not_equal, fill=1.0, base=0,
            pattern=[[-1, 128]], channel_multiplier=1)
        identb = const.tile([128, 128], BF16)
        nc.vector.tensor_copy(identb[:], ident[:])

        # dense-W constants on the scalar (act) HWDGE ring
        wg_t = const.tile([C, 5, 2 * H], BF16)
        wu_t = const.tile([C, 5, H], BF16)
        for j in range(5):
            nc.scalar.dma_start(wg_t[:, j, :],
                                d["Wg"].ap()[j * C:(j + 1) * C, :])
            nc.scalar.dma_start(wu_t[:, j, :],
                                d["Wu"].ap()[j * C:(j + 1) * C, :])
        bg_t = const.tile([2 * H, 1], F32)
        nc.scalar.dma_start(bg_t[:], d["bg"].ap())
        bu_t = const.tile([H, 1], F32)
        nc.scalar.dma_start(bu_t[:], d["bu"].ap())

        # ---- DRAM staging ----
        agA = [dram.tile([2 * NT * 128 * CA], BF16, name=f"agA{g}")
               for g in range(2)]
        agB = [dram.tile([2 * NT * 128 * CB], BF16, name=f"agB{g}")
               for g in range(2)]
        agAo = [dram.tile([NCORES * 2 * NT * 128 * CA], BF16,
                          name=f"agAo{g}", addr_space="Shared")
                for g in range(2)]
        agBo = [dram.tile([NCORES * 2 * NT * 128 * CB], BF16,
                          name=f"agBo{g}", addr_space="Shared")
                for g in range(2)]
        candA = dram.tile([NT * 128 * CA], BF16, name="candA")
        candB = dram.tile([NT * 128 * CB], BF16, name="candB")
        candAo = dram.tile([NCORES * NT * 128 * CA], BF16, name="candAo",
                           addr_space="Shared")
        candBo = dram.tile([NCORES * NT * 128 * CB], BF16, name="candBo",
                           addr_space="Shared")
        yt = [dram.tile([BC, NOWN], BF16, name=f"yt{i}") for i in range(4)]
        yt2 = [dram.tile([BC, NOWN], BF16, name=f"yt2_{i}") for i in range(4)]
        candT_dram = dram.tile([BC, NOWN], BF16)
        rt_dram = dram.tile([B, H, NOWN], BF16)

        def agA_own(g, s, t):
            o = ((s * NT + t) * 128) * CA
            return agA[g].opt()[o:o + 128 * CA].rearrange(
                "(p f) -> p f", f=CA)

        def agB_own(g, s, t):
            o = ((s * NT + t) * 128) * CB
            return agB[g].opt()[o:o + 128 * CB].rearrange(
                "(p f) -> p f", f=CB)

        def outA_q(g, s, q):
            # rank q's 4 m-tiles for support s: [p, t, f]
            o = ((q * 2 + s) * NT * 128) * CA
            return agAo[g].opt()[o:o + NT * 128 * CA].rearrange(
                "(t p f) -> p t f", p=128, f=CA)

        def outB_q(g, s, q):
            o = ((q * 2 + s) * NT * 128) * CB
            return agBo[g].opt()[o:o + NT * 128 * CB].rearrange(
                "(t p f) -> p t f", p=128, f=CB)

        def candA_own(t):
            o = t * 128 * CA
            return candA.opt()[o:o + 128 * CA].rearrange(
                "(p f) -> p f", f=CA)

        def candB_own(t):
            o = t * 128 * CB
            return candB.opt()[o:o + 128 * CB].rearrange(
                "(p f) -> p f", f=CB)

        def candAo_q(q):
            o = q * NT * 128 * CA
            return candAo.opt()[o:o + NT * 128 * CA].rearrange(
                "(t p f) -> p t f", p=128, f=CA)

        def candBo_q(q):
            o = q * NT * 128 * CB
            return candBo.opt()[o:o + NT * 128 * CB].rearrange(
                "(t p f) -> p t f", p=128, f=CB)

        def allgather(src, dst):
            nc.gpsimd.collective_compute(
                "AllGather", mybir.AluOpType.bypass, replica_groups=GROUP,
                ins=[src.opt()], outs=[dst.opt()])

        # ============ hop-1: node-major chunks + overlapped AG ============
        def emit_hop1_pair(pid, g, mov_main, mov_rag, yt_dst, stagger_T):
            """Y1_s[own rows, :] = A_s @ M for s in (0, 1); AG chunk A
            issued between the two main sweeps, chunk B after the ragged
            pass. Feature-major yt transposes are deferred (returned as a
            closure) so they land on the PE during the AG-B transfer."""
            kept = {}
            for hh in range(2):
                ps_m = {}
                for s in range(2):
                    for n in range(NT):
                        ps_m[(s, n)] = psum.tile(
                            [128, 512], F32, name=f"psm{pid}_{hh}{s}{n}",
                            tag="acc", bufs=8)
                for q in range(NQ):
                    if stagger_T and hh == 0 and q + 2 < NQ:
                        load_Tch(q + 2)
                    mv4 = sb_mov.tile([128, NT, 512], BF16,
                                      name=f"mv{pid}_{hh}_{q}", tag="mov",
                                      bufs=3)
                    nc.sync.dma_start(mv4[:], mov_main(q, hh))
                    for tt in range(NT):
                        m = q * NT + tt
                        for s in range(2):
                            for n in range(NT):
                                nc.tensor.matmul(
                                    ps_m[(s, n)][:],
                                    T_tile(s, m)[:, n * 128:(n + 1) * 128],
                                    mv4[:, tt, :], start=(m == 0),
                                    stop=(m == MT - 1))
                for s in range(2):
                    exhs = []
                    for n in range(NT):
                        exh = sb_ex.tile([128, 512], BF16,
                                         name=f"ex{pid}{hh}{s}{n}",
                                         tag="ex", bufs=17)
                        nc.vector.tensor_copy(exh[:], ps_m[(s, n)][:])
                        dst = (agA_own(g, s, n) if hh == 0
                               else agB_own(g, s, n)[:, 0:512])
                        nc.scalar.dma_start(dst, exh[:])
                        exhs.append(exh)
                    kept[(hh, s)] = exhs
                if hh == 0:
                    allgather(agA[g], agAo[g])

            # ragged pass (node-major staging feeds chunk B)
            ps_t = [psum.tile([RAG, NOWN], F32, name=f"pst{pid}{s}",
                              tag="acc", bufs=8) for s in range(2)]
            for q in range(NQ):
                mvr4 = sb_mov.tile([128, NT, RAG], BF16,
                                   name=f"mvr{pid}_{q}", tag="movr", bufs=3)
                nc.sync.dma_start(mvr4[:], mov_rag(q))
                for tt in range(NT):
                    m = q * NT + tt
                    for s in range(2):
                        nc.tensor.matmul(ps_t[s][:], mvr4[:, tt, :],
                                         T_tile(s, m), start=(m == 0),
                                         stop=(m == MT - 1))
            for s in range(2):
                rag_ex = sb_ex.tile([RAG, NOWN], BF16, name=f"rgex{pid}{s}",
                                    tag="ragex", bufs=2)
                nc.vector.tensor_copy(rag_ex[:], ps_t[s][:])
                nc.scalar.dma_start(yt_dst[s].opt()[MAIN:BC, :], rag_ex[:])
                for t in range(NT):
                    tp = psum.tile([128, RAG], BF16, name=f"rtp{pid}{s}",
                                   tag="acc", bufs=8)
                    nc.tensor.transpose(
                        tp[:], rag_ex[:, t * 128:(t + 1) * 128],
                        identb[0:RAG, 0:RAG])
                    rnm = sb_sm.tile([128, RAG], BF16, name=f"rnm{pid}{s}",
                                     tag="rnm", bufs=2)
                    nc.vector.tensor_copy(rnm[:], tp[:])
                    nc.scalar.dma_start(agB_own(g, s, t)[:, 512:544],
                                        rnm[:])
            allgather(agB[g], agBo[g])

            def deferred_yt():
                for s in range(2):
                    for hh in range(2):
                        for j in range(4):
                            st4 = sb_sm.tile([128, NOWN], BF16,
                                             name=f"st4{pid}", tag="st",
                                             bufs=3)
                            for n in range(NT):
                                tp = psum.tile([128, 128], BF16,
                                               name=f"tp{pid}", tag="acc",
                                               bufs=8)
                                nc.tensor.transpose(
                                    tp[:],
                                    kept[(hh, s)][n][:,
                                                     j * 128:(j + 1) * 128],
                                    identb[:])
                                nc.vector.tensor_copy(
                                    st4[:, n * 128:(n + 1) * 128], tp[:])
                            jj = hh * 4 + j
                            nc.scalar.dma_start(
                                yt_dst[s].opt()[jj * 128:(jj + 1) * 128, :],
                                st4[:])
            return deferred_yt

        # ======= hop-2 product: transposed form (feature-major out) =======
        def emit_hop2_pass(pid, s, g, part, yt_dst):
            """Y2raw^T[bc, own n] = (A_s @ Y1)^T for one column chunk."""
            wid = CA if part == 0 else CB
            ps = [psum.tile([128, NOWN], F32, name=f"ph2{pid}_{j}",
                            tag="acc", bufs=8) for j in range(4)]
            ps_r = None
            if part == 1:
                ps_r = psum.tile([RAG, NOWN], F32, name=f"ph2r{pid}",
                                 tag="acc", bufs=8)
            for q in range(NQ):
                mr4 = sb_mov.tile([128, NT, wid], BF16,
                                  name=f"mr{pid}_{q}",
                                  tag=("mov" if part == 0 else "movB"),
                                  bufs=(3 if part == 0 else 2))
                nc.sync.dma_start(
                    mr4[:], outA_q(g, s, q) if part == 0
                    else outB_q(g, s, q))
                for tt in range(NT):
                    m = q * NT + tt
                    for j in range(4):
                        nc.tensor.matmul(
                            ps[j][:],
                            mr4[:, tt, j * 128:(j + 1) * 128],
                            T_tile(s, m), start=(m == 0),
                            stop=(m == MT - 1))
                    if part == 1:
                        nc.tensor.matmul(ps_r[:], mr4[:, tt, 512:544],
                                         T_tile(s, m), start=(m == 0),
                                         stop=(m == MT - 1))
            for j in range(4):
                exh = sb_ex.tile([128, NOWN], BF16, name=f"h2ex{pid}_{j}",
                                 tag="ex", bufs=17)
                nc.vector.tensor_copy(exh[:], ps[j][:])
                jj = j if part == 0 else 4 + j
                nc.scalar.dma_start(
                    yt_dst.opt()[jj * 128:(jj + 1) * 128, :], exh[:])
            if part == 1:
                exr = sb_ex.tile([RAG, NOWN], BF16, name=f"h2exr{pid}",
                                 tag="ragex", bufs=2)
                nc.vector.tensor_copy(exr[:], ps_r[:])
                nc.scalar.dma_start(yt_dst.opt()[MAIN:BC, :], exr[:])

        # ======================= GCN 1 (gate) =======================
        def g1_main(q, hh):
            src = d["xs_main"].ap().rearrange("(q t p) f -> q p t f",
                                              p=128, t=NT)
            return src[q, :, :, hh * 512:(hh + 1) * 512]

        def g1_rag(q):
            src = d["xs_rag"].ap().rearrange("(q t) p f -> q p t f", t=NT)
            return src[q]

        dyt1 = emit_hop1_pair("g1h1", 0, g1_main, g1_rag,
                              (yt[0], yt[2]), True)
        dyt1()
        emit_hop2_pass("g1s0h2A", 0, 0, 0, yt[1])
        emit_hop2_pass("g1s1h2A", 1, 0, 0, yt[3])
        emit_hop2_pass("g1s0h2B", 0, 0, 1, yt[1])
        emit_hop2_pass("g1s1h2B", 1, 0, 1, yt[3])

        # ----- gate W-stage + candidate build, 4 batches per iter; the
        # candidate transpose/staging runs one iteration behind so the PE
        # queue alternates matmul-block, transpose-block without stalling.
        def gate_mm(pi):
            b0 = NB * pi
            xsT4 = sb_sm.tile([C, NB, NOWN], BF16, name="xsTb", tag="xsTb",
                              bufs=2)
            nc.sync.dma_start(
                xsT4[:],
                d["xsT_own"].ap()[b0 * C:(b0 + NB) * C, :]
                .rearrange("(b c) n -> c b n", b=NB))
            blocks = [xsT4]
            for j in range(4):
                bt = sb_sm.tile([C, NB, NOWN], BF16, name=f"blk{j}",
                                tag=f"blk{j}", bufs=2)
                nc.sync.dma_start(
                    bt[:],
                    yt[j].opt()[b0 * C:(b0 + NB) * C, :]
                    .rearrange("(b c) n -> c b n", b=NB))
                blocks.append(bt)
            zr_ps = [psum.tile([2 * H, NOWN], F32, name=f"zrps{b2}",
                               tag="acc", bufs=8) for b2 in range(NB)]
            for j in range(5):
                for b2 in range(NB):
                    nc.tensor.matmul(zr_ps[b2][:], wg_t[:, j, :],
                                     blocks[j][:, b2, :],
                                     start=(j == 0), stop=(j == 4))
            zr4 = sb_sm.tile([2 * H, NB, NOWN], BF16, name="zr", tag="zr",
                             bufs=2)
            for b2 in range(NB):
                nc.scalar.activation(zr4[:, b2, :], zr_ps[b2][:],
                                     AF.Sigmoid, bias=bg_t[:])
            nc.scalar.dma_start(
                rt_dram.opt()[b0:b0 + NB].rearrange("b (h n) -> h b n",
                                                    h=H),
                zr4[H:2 * H, :, :])
            cT4 = sb_sm.tile([C, NB, NOWN], BF16, name="cT", tag="cT",
                             bufs=2)
            nc.vector.tensor_mul(cT4[0:H, :, :], zr4[0:H, :, :],
                                 xsT4[0:H, :, :])
            nc.vector.tensor_copy(cT4[H:C, :, :], xsT4[H:C, :, :])
            nc.scalar.dma_start(
                candT_dram.opt()[b0 * C:(b0 + NB) * C, :]
                .rearrange("(b c) n -> c b n", b=NB), cT4[:])
            return cT4

        def cand_stage(pi, cT4):
            c0 = NB * pi * C
            hi = c0 + NB * C
            for t in range(NT):
                ct4 = sb_sm.tile([128, NB, C], BF16, name="ctnm",
                                 tag="ctnm", bufs=2)
                for b2 in range(NB):
                    tp = psum.tile([128, C], BF16, name="ctps", tag="acc",
                                   bufs=8)
                    nc.tensor.transpose(
                        tp[:], cT4[:, b2, t * 128:(t + 1) * 128],
                        identb[0:C, 0:C])
                    nc.vector.tensor_copy(ct4[:, b2, :], tp[:])
                flat = ct4[:].rearrange("p b c -> p (b c)")
                segs = []
                if c0 < CA:
                    e = min(hi, CA)
                    segs.append((candA_own(t)[:, c0:e], 0, e - c0))
                if hi > CA and c0 < MAIN:
                    s0 = max(c0, CA)
                    e = min(hi, MAIN)
                    segs.append((candB_own(t)[:, s0 - CA:e - CA],
                                 s0 - c0, e - s0))
                if hi > MAIN:
                    s0 = max(c0, MAIN)
                    segs.append((candB_own(t)[:, 512 + s0 - MAIN:
                                              512 + hi - MAIN],
                                 s0 - c0, hi - s0))
                for dst, off, w in segs:
                    nc.scalar.dma_start(dst, flat[:, off:off + w])

        prev = None
        for pi in range(B // NB):
            cur = gate_mm(pi)
            if prev is not None:
                cand_stage(pi - 1, prev)
                if pi - 1 == 1:
                    allgather(candA, candAo)
            prev = cur
        cand_stage(B // NB - 1, prev)
        allgather(candB, candBo)

        # ======================= GCN 2 (update) =======================
        def g2_main(q, hh):
            if hh == 0:
                return candAo_q(q)
            return candBo_q(q)[:, :, 0:512]

        def g2_rag(q):
            return candBo_q(q)[:, :, 512:544]

        dyt2 = emit_hop1_pair("g2h1", 1, g2_main, g2_rag,
                              (yt2[0], yt2[2]), False)
        dyt2()
        emit_hop2_pass("g2s0h2A", 0, 1, 0, yt2[1])
        emit_hop2_pass("g2s1h2A", 1, 1, 0, yt2[3])
        emit_hop2_pass("g2s0h2B", 0, 1, 1, yt2[1])
        emit_hop2_pass("g2s1h2B", 1, 1, 1, yt2[3])

        # update W-stage + final combine, 4 batches per iter
        for pi in range(B // NB):
            b0 = NB * pi
            cT4b = sb_sm.tile([C, NB, NOWN], BF16, name="cTb", tag="xsTb",
                              bufs=2)
            nc.sync.dma_start(
                cT4b[:],
                candT_dram.opt()[b0 * C:(b0 + NB) * C, :]
                .rearrange("(b c) n -> c b n", b=NB))
            blocks = [cT4b]
            for j in range(4):
                bt = sb_sm.tile([C, NB, NOWN], BF16, name=f"ublk{j}",
                                tag=f"blk{j}", bufs=2)
                nc.sync.dma_start(
                    bt[:],
                    yt2[j].opt()[b0 * C:(b0 + NB) * C, :]
                    .rearrange("(b c) n -> c b n", b=NB))
                blocks.append(bt)
            hc_ps = [psum.tile([H, NOWN], F32, name=f"hcps{b2}", tag="acc",
                               bufs=8) for b2 in range(NB)]
            for j in range(5):
                for b2 in range(NB):
                    nc.tensor.matmul(hc_ps[b2][:], wu_t[:, j, :],
                                     blocks[j][:, b2, :],
                                     start=(j == 0), stop=(j == 4))
            hc4 = sb_sm.tile([H, NB, NOWN], BF16, name="hc", tag="zr",
                             bufs=2)
            for b2 in range(NB):
                nc.scalar.activation(hc4[:, b2, :], hc_ps[b2][:], AF.Tanh,
                                     bias=bu_t[:])

            # out = hc + r * (state - hc); state rows are xsT_own[0:H]
            stT4 = sb_sm.tile([H, NB, NOWN], BF16, name="stTu", tag="stg",
                              bufs=2)
            nc.sync.dma_start(
                stT4[:],
                d["xsT_own"].ap()[b0 * C:(b0 + NB) * C, :]
                .rearrange("(b c) n -> c b n", b=NB)[0:H])
            rT4 = sb_sm.tile([H, NB, NOWN], BF16, name="rT", tag="rT",
                             bufs=2)
            nc.sync.dma_start(
                rT4[:],
                rt_dram.opt()[b0:b0 + NB].rearrange("b (h n) -> h b n",
                                                    h=H))
            tmp4 = sb_sm.tile([H, NB, NOWN], BF16, name="tmp", tag="tmp",
                              bufs=2)
            nc.vector.tensor_sub(tmp4[:], stT4[:], hc4[:])
            nc.vector.tensor_mul(tmp4[:], rT4[:], tmp4[:])
            ot4 = sb_sm.tile([H, NB, NOWN], F32, name="ot", tag="ot",
                             bufs=1)
            nc.vector.tensor_add(ot4[:], hc4[:], tmp4[:])
            nc.scalar.dma_start(
                d["outT"].ap()[b0:b0 + NB].rearrange("b h n -> h b n"),
                ot4[:])


def prepare_in_maps(x, state, support0, support1, W_gate, b_gate,
                    W_update, b_update):
    BFNP = mybir.dt.np(BF16)
    xs = np.concatenate([x, state], axis=-1)          # [B, N, C]
    xs_nm = np.ascontiguousarray(
        xs.transpose(1, 0, 2).reshape(N, BC)).astype(BFNP)
    # feature-major input for W / elementwise uses [state(64); x(2)] rows
    sx_nm = np.ascontiguousarray(
        np.concatenate([state, x], axis=-1)
        .transpose(1, 0, 2).reshape(N, BC)).astype(np.float32)
    perm = np.r_[DIN:C, 0:DIN]                 # [x, state] -> [state, x]

    # fold the Chebyshev combination x2 = 2*A@x1 - x0 into W:
    # W0 -= (W2 + W4); W2 *= 2; W4 *= 2  (per 66-row block)
    def fold(W):
        Wf = np.ascontiguousarray(W, dtype=np.float32).copy()
        Wf[0:C] -= Wf[2 * C:3 * C] + Wf[4 * C:5 * C]
        Wf[2 * C:3 * C] *= 2.0
        Wf[4 * C:5 * C] *= 2.0
        return Wf

    Wg_dev = fold(W_gate)
    Wg_dev[0:C] = Wg_dev[0:C][perm]            # only the X-block reads xsT
    Wu_dev = fold(W_update)
    for j in range(5):                         # all of cand's blocks permute
        Wu_dev[j * C:(j + 1) * C] = Wu_dev[j * C:(j + 1) * C][perm]
    Wg_dev = Wg_dev.astype(BFNP)
    Wu_dev = Wu_dev.astype(BFNP)

    xs_main = np.ascontiguousarray(xs_nm[:, :MAIN])
    xs_rag = np.ascontiguousarray(xs_nm[:, MAIN:]).reshape(MT, 128, RAG)
    bg = np.ascontiguousarray(b_gate, dtype=np.float32).reshape(2 * H, 1)
    bu = np.ascontiguousarray(b_update, dtype=np.float32).reshape(H, 1)
    s0b = np.asarray(support0, dtype=np.float32).astype(BFNP)
    s1b = np.asarray(support1, dtype=np.float32).astype(BFNP)

    in_maps = []
    for r in range(NCORES):
        n0 = r * NOWN
        in_maps.append({
            "Ts": np.ascontiguousarray(
                np.stack([s0b[n0:n0 + NOWN, :].T,
                          s1b[n0:n0 + NOWN, :].T])),
            "xs_main": xs_main,
            "xs_rag": xs_rag,
            "xsT_own": np.ascontiguousarray(
                sx_nm[n0:n0 + NOWN].T).astype(BFNP),
            "Wg": Wg_dev, "bg": bg, "Wu": Wu_dev, "bu": bu,
        })
    return in_maps


def assemble_output(results):
    out = np.empty((B, N, H), dtype=np.float32)
    for r in range(NCORES):
        n0 = r * NOWN
        out[:, n0:n0 + NOWN, :] = results[r]["outT"].transpose(0, 2, 1)
    return out


def get_nc():
    if "nc" not in _NC_CACHE:
        _NC_CACHE["nc"] = build_nc()
    return _NC_CACHE["nc"]


def kernel(x, state, support0, support1, W_gate, b_gate, W_update, b_update):
    nc = get_nc()
    in_maps = prepare_in_maps(x, state, support0, support1,
                              W_gate, b_gate, W_update, b_update)
    prev = os.environ.get("BASS_NEVER_TRACE")
    os.environ["BASS_NEVER_TRACE"] = "1"
    try:
        res = run_bass_kernel_spmd(nc, in_maps, list(range(NCORES)),
                                   trace=False)
    finally:
        if prev is None:
            os.environ.pop("BASS_NEVER_TRACE", None)
        else:
            os.environ["BASS_NEVER_TRACE"] = prev
    return assemble_output(res.results)


# revision 15
# speedup vs baseline: 1.7409x; 1.0130x over previous
"""DCGRU cell (nn_DCGRUCell) Trainium2 Bass kernel, 8 NeuronCores.

Sharding: node dimension N=4096 split 8 ways (512 rows/core); supports are
fed host-transposed (T = A^T), held resident in SBUF as bf16. Hop-1
diffusion products are computed node-major and AllGathered in two
column-chunks per GCN (A: bc cols 0-511; B: cols 512-1023 + 32 ragged) so
the gathers overlap hop-1/hop-2 compute. Hop-2 products are computed
directly in transposed (feature-major) form. All matmul operands are bf16
(PSUM fp32); the Chebyshev combination x2 = 2*A@x1 - x0 is folded into the
dense W matrices host-side. Moving operands are loaded 4 m-tiles per DMA;
the dense W stage processes batches 4 at a time with the candidate
transposes software-pipelined one iteration behind the matmuls. DMA issue
is split across the two HWDGE rings (sync=loads, scalar=stores).

kernel(**inputs) takes the FULL inputs from reference.setup_inputs() and
returns the FULL [16, 4096, 64] float32 output.
"""
import os
import numpy as np

import concourse.bass as bass
import concourse.mybir as mybir
import concourse.tile as tile
from concourse import bacc
from concourse.bass_utils import run_bass_kernel_spmd

F32 = mybir.dt.float32
BF16 = mybir.dt.bfloat16
AF = mybir.ActivationFunctionType

NCORES = 8
B, N, H, DIN = 16, 4096, 64, 2
C = DIN + H                 # 66 features per batch into each GCN
BC = B * C                  # 1056
NOWN = N // NCORES          # 512 rows per core
NT = NOWN // 128            # 4 n-tiles per core
MT = N // 128               # 32 m-tiles (contraction)
NQ = MT // NT               # 8 rank-blocks of 4 m-tiles
CA = 512                    # chunk A: bc columns 0:512
CB = 544                    # chunk B: bc columns 512:1024 + 32 ragged
MAIN = 1024
RAG = BC - MAIN             # 32 ragged columns
NB = 4                      # batches per W-stage iteration
GROUP = [list(range(NCORES))]

_NC_CACHE = {}


def build_nc():
    nc = bacc.Bacc("TRN2", target_bir_lowering=False, debug=False,
                   num_devices=NCORES)

    d = {}
    d["Ts"] = nc.dram_tensor("Ts", [2, N, NOWN], BF16, kind="ExternalInput")
    d["xs_main"] = nc.dram_tensor("xs_main", [N, MAIN], BF16,
                                  kind="ExternalInput")
    d["xs_rag"] = nc.dram_tensor("xs_rag", [MT, 128, RAG], BF16,
                                 kind="ExternalInput")
    d["xsT_own"] = nc.dram_tensor("xsT_own", [BC, NOWN], BF16,
                                  kind="ExternalInput")
    d["Wg"] = nc.dram_tensor("Wg", [5 * C, 2 * H], BF16, kind="ExternalInput")
    d["bg"] = nc.dram_tensor("bg", [2 * H, 1], F32, kind="ExternalInput")
    d["Wu"] = nc.dram_tensor("Wu", [5 * C, H], BF16, kind="ExternalInput")
    d["bu"] = nc.dram_tensor("bu", [H, 1], F32, kind="ExternalInput")
    d["outT"] = nc.dram_tensor("outT", [B, H, NOWN], F32,
                               kind="ExternalOutput")

    with tile.TileContext(nc) as tc:
        _emit(nc, tc, d)
    nc.compile()
    return nc


def _emit(nc, tc, d):
    import contextlib
    stack = contextlib.ExitStack()
    with stack:
        const = stack.enter_context(tc.tile_pool(name="const", bufs=1))
        sb_ex = stack.enter_context(tc.tile_pool(name="ex", bufs=1))
        sb_mov = stack.enter_context(tc.tile_pool(name="mov", bufs=1))
        sb_sm = stack.enter_context(tc.tile_pool(name="small", bufs=1))
        dram = stack.enter_context(
            tc.tile_pool(name="dram", bufs=1, space="DRAM"))
        psum = stack.enter_context(
            tc.tile_pool(name="psum", bufs=1, space="PSUM"))

        # ---- resident support tiles (loaded staggered in first sweep) ----
        Tch = {}
        for s in range(2):
            for k in range(NQ):
                Tch[(s, k)] = const.tile([128, NT, 512], BF16,
                                         name=f"T{s}_{k}")

        def load_Tch(k):
            for s in range(2):
                ts = d["Ts"].ap()[s].rearrange("(t p) n -> p t n", p=128)
                nc.scalar.dma_start(Tch[(s, k)][:],
                                    ts[:, k * NT:(k + 1) * NT, :])

        load_Tch(0)
        load_Tch(1)

        def T_tile(s, m):
            return Tch[(s, m // NT)][:, m % NT, :]

        ident = const.tile([128, 128], F32)
        nc.gpsimd.memset(ident[:], 0.0)
        nc.gpsimd.affine_select(
            out=ident[:], in_=ident[:],
            compare_op=mybir.AluOpType.not_equal, fill=1.0, base=0,
            pattern=[[-1, 128]], channel_multiplier=1)
        identb = const.tile([128, 128], BF16)
        nc.vector.tensor_copy(identb[:], ident[:])

        # dense-W constants on the scalar (act) HWDGE ring
        wg_t = const.tile([C, 5, 2 * H], BF16)
        wu_t = const.tile([C, 5, H], BF16)
        for j in range(5):
            nc.scalar.dma_start(wg_t[:, j, :],
                                d["Wg"].ap()[j * C:(j + 1) * C, :])
            nc.scalar.dma_start(wu_t[:, j, :],
                                d["Wu"].ap()[j * C:(j + 1) * C, :])
        bg_t = const.tile([2 * H, 1], F32)
        nc.scalar.dma_start(bg_t[:], d["bg"].ap())
        bu_t = const.tile([H, 1], F32)
        nc.scalar.dma_start(bu_t[:], d["bu"].ap())

        # ---- DRAM staging ----
        agA = [dram.tile([2 * NT * 128 * CA], BF16, name=f"agA{g}")
               for g in range(2)]
        agB = [dram.tile([2 * NT * 128 * CB], BF16, name=f"agB{g}")
               for g in range(2)]
        agAo = [dram.tile([NCORES * 2 * NT * 128 * CA], BF16,
                          name=f"agAo{g}", addr_space="Shared")
                for g in range(2)]
        agBo = [dram.tile([NCORES * 2 * NT * 128 * CB], BF16,
                          name=f"agBo{g}", addr_space="Shared")
                for g in range(2)]
        candA = dram.tile([NT * 128 * CA], BF16, name="candA")
        candB = dram.tile([NT * 128 * CB], BF16, name="candB")
        candAo = dram.tile([NCORES * NT * 128 * CA], BF16, name="candAo",
                           addr_space="Shared")
        candBo = dram.tile([NCORES * NT * 128 * CB], BF16, name="candBo",
                           addr_space="Shared")
        yt = [dram.tile([BC, NOWN], BF16, name=f"yt{i}") for i in range(4)]
        yt2 = [dram.tile([BC, NOWN], BF16, name=f"yt2_{i}") for i in range(4)]
        candT_dram = dram.tile([BC, NOWN], BF16)
        rt_dram = dram.tile([B, H, NOWN], BF16)

        def agA_own(g, s, t):
            o = ((s * NT + t) * 128) * CA
            return agA[g].opt()[o:o + 128 * CA].rearrange(
                "(p f) -> p f", f=CA)

        def agB_own(g, s, t):
            o = ((s * NT + t) * 128) * CB
            return agB[g].opt()[o:o + 128 * CB].rearrange(
                "(p f) -> p f", f=CB)

        def outA_q(g, s, q):
            # rank q's 4 m-tiles for support s: [p, t, f]
            o = ((q * 2 + s) * NT * 128) * CA
            return agAo[g].opt()[o:o + NT * 128 * CA].rearrange(
                "(t p f) -> p t f", p=128, f=CA)

        def outB_q(g, s, q):
            o = ((q * 2 + s) * NT * 128) * CB
            return agBo[g].opt()[o:o + NT * 128 * CB].rearrange(
                "(t p f) -> p t f", p=128, f=CB)

        def candA_own(t):
            o = t * 128 * CA
            return candA.opt()[o:o + 128 * CA].rearrange(
                "(p f) -> p f", f=CA)

        def candB_own(t):
            o = t * 128 * CB
            return candB.opt()[o:o + 128 * CB].rearrange(
                "(p f) -> p f", f=CB)

        def candAo_q(q):
            o = q * NT * 128 * CA
            return candAo.opt()[o:o + NT * 128 * CA].rearrange(
                "(t p f) -> p t f", p=128, f=CA)

        def candBo_q(q):
            o = q * NT * 128 * CB
            return candBo.opt()[o:o + NT * 128 * CB].rearrange(
                "(t p f) -> p t f", p=128, f=CB)

        def allgather(src, dst):
            nc.gpsimd.collective_compute(
                "AllGather", mybir.AluOpType.bypass, replica_groups=GROUP,
                ins=[src.opt()], outs=[dst.opt()])

        # ============ hop-1: node-major chunks + overlapped AG ============
        def emit_hop1_pair(pid, g, mov_main, mov_rag, yt_dst, stagger_T,
                           ld):
            """Y1_s[own rows, :] = A_s @ M for s in (0, 1); AG chunk A
            issued between the two main sweeps, chunk B after the ragged
            pass. Feature-major yt transposes are deferred (returned as a
            closure) so they land on the PE during the AG-B transfer.
            `ld` is the engine issuing moving-operand loads: for GCN1 the
            scalar ring (loads are AG-independent and must not sit behind
            hop-2's AG-gated loads on the sync ring); for GCN2 the sync
            ring (its loads are AG-gated like everything behind them)."""
            kept = {}
            preloaded = {}
            for hh in range(2):
                ps_m = {}
                for s in range(2):
                    for n in range(NT):
                        ps_m[(s, n)] = psum.tile(
                            [128, 512], F32, name=f"psm{pid}_{hh}{s}{n}",
                            tag="acc", bufs=8)
                for q in range(NQ):
                    if stagger_T and hh == 0 and q + 2 < NQ:
                        load_Tch(q + 2)
                    if (hh, q) in preloaded:
                        mv4 = preloaded.pop((hh, q))
                    else:
                        mv4 = sb_mov.tile([128, NT, 512], BF16,
                                          name=f"mv{pid}_{hh}_{q}",
                                          tag="mov", bufs=4)
                        ld.dma_start(mv4[:], mov_main(q, hh))
                    for tt in range(NT):
                        m = q * NT + tt
                        for s in range(2):
                            for n in range(NT):
                                nc.tensor.matmul(
                                    ps_m[(s, n)][:],
                                    T_tile(s, m)[:, n * 128:(n + 1) * 128],
                                    mv4[:, tt, :], start=(m == 0),
                                    stop=(m == MT - 1))
                if hh == 0:
                    # prefetch the next sweep's first tiles ahead of the
                    # staging stores so the ring never idles the PE at the
                    # sweep boundary
                    for qq in range(2):
                        mv4p = sb_mov.tile([128, NT, 512], BF16,
                                           name=f"mv{pid}_1_{qq}",
                                           tag="mov", bufs=4)
                        ld.dma_start(mv4p[:], mov_main(qq, 1))
                        preloaded[(1, qq)] = mv4p
                for s in range(2):
                    exhs = []
                    for n in range(NT):
                        exh = sb_ex.tile([128, 512], BF16,
                                         name=f"ex{pid}{hh}{s}{n}",
                                         tag="ex", bufs=16)
                        nc.vector.tensor_copy(exh[:], ps_m[(s, n)][:])
                        dst = (agA_own(g, s, n) if hh == 0
                               else agB_own(g, s, n)[:, 0:512])
                        nc.scalar.dma_start(dst, exh[:])
                        exhs.append(exh)
                    kept[(hh, s)] = exhs
                if hh == 0:
                    allgather(agA[g], agAo[g])

            # ragged pass (node-major staging feeds chunk B)
            ps_t = [psum.tile([RAG, NOWN], F32, name=f"pst{pid}{s}",
                              tag="acc", bufs=8) for s in range(2)]
            for q in range(NQ):
                mvr4 = sb_mov.tile([128, NT, RAG], BF16,
                                   name=f"mvr{pid}_{q}", tag="movr", bufs=3)
                ld.dma_start(mvr4[:], mov_rag(q))
                for tt in range(NT):
                    m = q * NT + tt
                    for s in range(2):
                        nc.tensor.matmul(ps_t[s][:], mvr4[:, tt, :],
                                         T_tile(s, m), start=(m == 0),
                                         stop=(m == MT - 1))
            for s in range(2):
                rag_ex = sb_ex.tile([RAG, NOWN], BF16, name=f"rgex{pid}{s}",
                                    tag="ragex", bufs=2)
                nc.vector.tensor_copy(rag_ex[:], ps_t[s][:])
                nc.scalar.dma_start(yt_dst[s].opt()[MAIN:BC, :], rag_ex[:])
                for t in range(NT):
                    tp = psum.tile([128, RAG], BF16, name=f"rtp{pid}{s}",
                                   tag="acc", bufs=8)
                    nc.tensor.transpose(
                        tp[:], rag_ex[:, t * 128:(t + 1) * 128],
                        identb[0:RAG, 0:RAG])
                    rnm = sb_sm.tile([128, RAG], BF16, name=f"rnm{pid}{s}",
                                     tag="rnm", bufs=2)
                    nc.vector.tensor_copy(rnm[:], tp[:])
                    nc.scalar.dma_start(agB_own(g, s, t)[:, 512:544],
                                        rnm[:])
            allgather(agB[g], agBo[g])

            def deferred_yt():
                for s in range(2):
                    for hh in range(2):
                        for j in range(4):
                            st4 = sb_sm.tile([128, NOWN], BF16,
                                             name=f"st4{pid}", tag="st",
                                             bufs=3)
                            for n in range(NT):
                                tp = psum.tile([128, 128], BF16,
                                               name=f"tp{pid}", tag="acc",
                                               bufs=8)
                                nc.tensor.transpose(
                                    tp[:],
                                    kept[(hh, s)][n][:,
                                                     j * 128:(j + 1) * 128],
                                    identb[:])
                                nc.vector.tensor_copy(
                                    st4[:, n * 128:(n + 1) * 128], tp[:])
                            jj = hh * 4 + j
                            nc.scalar.dma_start(
                                yt_dst[s].opt()[jj * 128:(jj + 1) * 128, :],
                                st4[:])
            return deferred_yt

        # ======= hop-2 product: transposed form (feature-major out) =======
        def emit_hop2_pass(pid, s, g, part, yt_dst):
            """Y2raw^T[bc, own n] = (A_s @ Y1)^T for one column chunk."""
            wid = CA if part == 0 else CB
            ps = [psum.tile([128, NOWN], F32, name=f"ph2{pid}_{j}",
                            tag="acc", bufs=8) for j in range(4)]
            ps_r = None
            if part == 1:
                ps_r = psum.tile([RAG, NOWN], F32, name=f"ph2r{pid}",
                                 tag="acc", bufs=8)
            for q in range(NQ):
                mr4 = sb_mov.tile([128, NT, wid], BF16,
                                  name=f"mr{pid}_{q}",
                                  tag=("mov" if part == 0 else "movB"),
                                  bufs=(4 if part == 0 else 2))
                nc.sync.dma_start(
                    mr4[:], outA_q(g, s, q) if part == 0
                    else outB_q(g, s, q))
                for tt in range(NT):
                    m = q * NT + tt
                    for j in range(4):
                        nc.tensor.matmul(
                            ps[j][:],
                            mr4[:, tt, j * 128:(j + 1) * 128],
                            T_tile(s, m), start=(m == 0),
                            stop=(m == MT - 1))
                    if part == 1:
                        nc.tensor.matmul(ps_r[:], mr4[:, tt, 512:544],
                                         T_tile(s, m), start=(m == 0),
                                         stop=(m == MT - 1))
            for j in range(4):
                exh = sb_ex.tile([128, NOWN], BF16, name=f"h2ex{pid}_{j}",
                                 tag="ex", bufs=16)
                nc.vector.tensor_copy(exh[:], ps[j][:])
                jj = j if part == 0 else 4 + j
                nc.scalar.dma_start(
                    yt_dst.opt()[jj * 128:(jj + 1) * 128, :], exh[:])
            if part == 1:
                exr = sb_ex.tile([RAG, NOWN], BF16, name=f"h2exr{pid}",
                                 tag="ragex", bufs=2)
                nc.vector.tensor_copy(exr[:], ps_r[:])
                nc.scalar.dma_start(yt_dst.opt()[MAIN:BC, :], exr[:])

        # ======================= GCN 1 (gate) =======================
        def g1_main(q, hh):
            src = d["xs_main"].ap().rearrange("(q t p) f -> q p t f",
                                              p=128, t=NT)
            return src[q, :, :, hh * 512:(hh + 1) * 512]

        def g1_rag(q):
            src = d["xs_rag"].ap().rearrange("(q t) p f -> q p t f", t=NT)
            return src[q]

        dyt1 = emit_hop1_pair("g1h1", 0, g1_main, g1_rag,
                              (yt[0], yt[2]), True, nc.scalar)
        dyt1()
        emit_hop2_pass("g1s0h2A", 0, 0, 0, yt[1])
        emit_hop2_pass("g1s1h2A", 1, 0, 0, yt[3])
        emit_hop2_pass("g1s0h2B", 0, 0, 1, yt[1])
        emit_hop2_pass("g1s1h2B", 1, 0, 1, yt[3])

        # ----- gate W-stage + candidate build, 4 batches per iter; the
        # candidate transpose/staging runs one iteration behind so the PE
        # queue alternates matmul-block, transpose-block without stalling.
        def gate_mm(pi):
            b0 = NB * pi
            xsT4 = sb_sm.tile([C, NB, NOWN], BF16, name="xsTb", tag="xsTb",
                              bufs=2)
            nc.sync.dma_start(
                xsT4[:],
                d["xsT_own"].ap()[b0 * C:(b0 + NB) * C, :]
                .rearrange("(b c) n -> c b n", b=NB))
            blocks = [xsT4]
            for j in range(4):
                bt = sb_sm.tile([C, NB, NOWN], BF16, name=f"blk{j}",
                                tag=f"blk{j}", bufs=2)
                nc.sync.dma_start(
                    bt[:],
                    yt[j].opt()[b0 * C:(b0 + NB) * C, :]
                    .rearrange("(b c) n -> c b n", b=NB))
                blocks.append(bt)
            zr_ps = [psum.tile([2 * H, NOWN], F32, name=f"zrps{b2}",
                               tag="acc", bufs=8) for b2 in range(NB)]
            for j in range(5):
                for b2 in range(NB):
                    nc.tensor.matmul(zr_ps[b2][:], wg_t[:, j, :],
                                     blocks[j][:, b2, :],
                                     start=(j == 0), stop=(j == 4))
            zr4 = sb_sm.tile([2 * H, NB, NOWN], BF16, name="zr", tag="zr",
                             bufs=2)
            for b2 in range(NB):
                nc.scalar.activation(zr4[:, b2, :], zr_ps[b2][:],
                                     AF.Sigmoid, bias=bg_t[:])
            nc.scalar.dma_start(
                rt_dram.opt()[b0:b0 + NB].rearrange("b (h n) -> h b n",
                                                    h=H),
                zr4[H:2 * H, :, :])
            cT4 = sb_sm.tile([C, NB, NOWN], BF16, name="cT", tag="cT",
                             bufs=2)
            nc.vector.tensor_mul(cT4[0:H, :, :], zr4[0:H, :, :],
                                 xsT4[0:H, :, :])
            nc.vector.tensor_copy(cT4[H:C, :, :], xsT4[H:C, :, :])
            nc.scalar.dma_start(
                candT_dram.opt()[b0 * C:(b0 + NB) * C, :]
                .rearrange("(b c) n -> c b n", b=NB), cT4[:])
            return cT4

        def cand_stage(pi, cT4):
            c0 = NB * pi * C
            hi = c0 + NB * C
            for t in range(NT):
                ct4 = sb_sm.tile([128, NB, C], BF16, name="ctnm",
                                 tag="ctnm", bufs=2)
                for b2 in range(NB):
                    tp = psum.tile([128, C], BF16, name="ctps", tag="acc",
                                   bufs=8)
                    nc.tensor.transpose(
                        tp[:], cT4[:, b2, t * 128:(t + 1) * 128],
                        identb[0:C, 0:C])
                    nc.vector.tensor_copy(ct4[:, b2, :], tp[:])
                flat = ct4[:].rearrange("p b c -> p (b c)")
                segs = []
                if c0 < CA:
                    e = min(hi, CA)
                    segs.append((candA_own(t)[:, c0:e], 0, e - c0))
                if hi > CA and c0 < MAIN:
                    s0 = max(c0, CA)
                    e = min(hi, MAIN)
                    segs.append((candB_own(t)[:, s0 - CA:e - CA],
                                 s0 - c0, e - s0))
                if hi > MAIN:
                    s0 = max(c0, MAIN)
                    segs.append((candB_own(t)[:, 512 + s0 - MAIN:
                                              512 + hi - MAIN],
                                 s0 - c0, hi - s0))
                for dst, off, w in segs:
                    nc.scalar.dma_start(dst, flat[:, off:off + w])

        prev = None
        for pi in range(B // NB):
            cur = gate_mm(pi)
            if prev is not None:
                cand_stage(pi - 1, prev)
                if pi - 1 == 1:
                    allgather(candA, candAo)
            prev = cur
        cand_stage(B // NB - 1, prev)
        allgather(candB, candBo)

        # ======================= GCN 2 (update) =======================
        def g2_main(q, hh):
            if hh == 0:
                return candAo_q(q)
            return candBo_q(q)[:, :, 0:512]

        def g2_rag(q):
            return candBo_q(q)[:, :, 512:544]

        dyt2 = emit_hop1_pair("g2h1", 1, g2_main, g2_rag,
                              (yt2[0], yt2[2]), False, nc.sync)
        dyt2()
        emit_hop2_pass("g2s0h2A", 0, 1, 0, yt2[1])
        emit_hop2_pass("g2s1h2A", 1, 1, 0, yt2[3])
        emit_hop2_pass("g2s0h2B", 0, 1, 1, yt2[1])
        emit_hop2_pass("g2s1h2B", 1, 1, 1, yt2[3])

        # update W-stage + final combine, 4 batches per iter
        for pi in range(B // NB):
            b0 = NB * pi
            cT4b = sb_sm.tile([C, NB, NOWN], BF16, name="cTb", tag="xsTb",
                              bufs=2)
            nc.sync.dma_start(
                cT4b[:],
                candT_dram.opt()[b0 * C:(b0 + NB) * C, :]
                .rearrange("(b c) n -> c b n", b=NB))
            blocks = [cT4b]
            for j in range(4):
                bt = sb_sm.tile([C, NB, NOWN], BF16, name=f"ublk{j}",
                                tag=f"blk{j}", bufs=2)
                nc.sync.dma_start(
                    bt[:],
                    yt2[j].opt()[b0 * C:(b0 + NB) * C, :]
                    .rearrange("(b c) n -> c b n", b=NB))
                blocks.append(bt)
            hc_ps = [psum.tile([H, NOWN], F32, name=f"hcps{b2}", tag="acc",
                               bufs=8) for b2 in range(NB)]
            for j in range(5):
                for b2 in range(NB):
                    nc.tensor.matmul(hc_ps[b2][:], wu_t[:, j, :],
                                     blocks[j][:, b2, :],
                                     start=(j == 0), stop=(j == 4))
            hc4 = sb_sm.tile([H, NB, NOWN], BF16, name="hc", tag="zr",
                             bufs=2)
            for b2 in range(NB):
                nc.scalar.activation(hc4[:, b2, :], hc_ps[b2][:], AF.Tanh,
                                     bias=bu_t[:])

            # out = hc + r * (state - hc); state rows are xsT_own[0:H]
            stT4 = sb_sm.tile([H, NB, NOWN], BF16, name="stTu", tag="stg",
                              bufs=2)
            nc.sync.dma_start(
                stT4[:],
                d["xsT_own"].ap()[b0 * C:(b0 + NB) * C, :]
                .rearrange("(b c) n -> c b n", b=NB)[0:H])
            rT4 = sb_sm.tile([H, NB, NOWN], BF16, name="rT", tag="rT",
                             bufs=2)
            nc.sync.dma_start(
                rT4[:],
                rt_dram.opt()[b0:b0 + NB].rearrange("b (h n) -> h b n",
                                                    h=H))
            tmp4 = sb_sm.tile([H, NB, NOWN], BF16, name="tmp", tag="tmp",
                              bufs=2)
            nc.vector.tensor_sub(tmp4[:], stT4[:], hc4[:])
            nc.vector.tensor_mul(tmp4[:], rT4[:], tmp4[:])
            ot4 = sb_sm.tile([H, NB, NOWN], F32, name="ot", tag="ot",
                             bufs=1)
            nc.vector.tensor_add(ot4[:], hc4[:], tmp4[:])
            nc.scalar.dma_start(
                d["outT"].ap()[b0:b0 + NB].rearrange("b h n -> h b n"),
                ot4[:])


def prepare_in_maps(x, state, support0, support1, W_gate, b_gate,
                    W_update, b_update):
    BFNP = mybir.dt.np(BF16)
    xs = np.concatenate([x, state], axis=-1)          # [B, N, C]
    xs_nm = np.ascontiguousarray(
        xs.transpose(1, 0, 2).reshape(N, BC)).astype(BFNP)
    # feature-major input for W / elementwise uses [state(64); x(2)] rows
    sx_nm = np.ascontiguousarray(
        np.concatenate([state, x], axis=-1)
        .transpose(1, 0, 2).reshape(N, BC)).astype(np.float32)
    perm = np.r_[DIN:C, 0:DIN]                 # [x, state] -> [state, x]

    # fold the Chebyshev combination x2 = 2*A@x1 - x0 into W:
    # W0 -= (W2 + W4); W2 *= 2; W4 *= 2  (per 66-row block)
    def fold(W):
        Wf = np.ascontiguousarray(W, dtype=np.float32).copy()
        Wf[0:C] -= Wf[2 * C:3 * C] + Wf[4 * C:5 * C]
        Wf[2 * C:3 * C] *= 2.0
        Wf[4 * C:5 * C] *= 2.0
        return Wf

    Wg_dev = fold(W_gate)
    Wg_dev[0:C] = Wg_dev[0:C][perm]            # only the X-block reads xsT
    Wu_dev = fold(W_update)
    for j in range(5):                         # all of cand's blocks permute
        Wu_dev[j * C:(j + 1) * C] = Wu_dev[j * C:(j + 1) * C][perm]
    Wg_dev = Wg_dev.astype(BFNP)
    Wu_dev = Wu_dev.astype(BFNP)

    xs_main = np.ascontiguousarray(xs_nm[:, :MAIN])
    xs_rag = np.ascontiguousarray(xs_nm[:, MAIN:]).reshape(MT, 128, RAG)
    bg = np.ascontiguousarray(b_gate, dtype=np.float32).reshape(2 * H, 1)
    bu = np.ascontiguousarray(b_update, dtype=np.float32).reshape(H, 1)
    s0b = np.asarray(support0, dtype=np.float32).astype(BFNP)
    s1b = np.asarray(support1, dtype=np.float32).astype(BFNP)

    in_maps = []
    for r in range(NCORES):
        n0 = r * NOWN
        in_maps.append({
            "Ts": np.ascontiguousarray(
                np.stack([s0b[n0:n0 + NOWN, :].T,
                          s1b[n0:n0 + NOWN, :].T])),
            "xs_main": xs_main,
            "xs_rag": xs_rag,
            "xsT_own": np.ascontiguousarray(
                sx_nm[n0:n0 + NOWN].T).astype(BFNP),
            "Wg": Wg_dev, "bg": bg, "Wu": Wu_dev, "bu": bu,
        })
    return in_maps


def assemble_output(results):
    out = np.empty((B, N, H), dtype=np.float32)
    for r in range(NCORES):
        n0 = r * NOWN
        out[:, n0:n0 + NOWN, :] = results[r]["outT"].transpose(0, 2, 1)
    return out


def get_nc():
    if "nc" not in _NC_CACHE:
        _NC_CACHE["nc"] = build_nc()
    return _NC_CACHE["nc"]


def kernel(x, state, support0, support1, W_gate, b_gate, W_update, b_update):
    nc = get_nc()
    in_maps = prepare_in_maps(x, state, support0, support1,
                              W_gate, b_gate, W_update, b_update)
    prev = os.environ.get("BASS_NEVER_TRACE")
    os.environ["BASS_NEVER_TRACE"] = "1"
    try:
        res = run_bass_kernel_spmd(nc, in_maps, list(range(NCORES)),
                                   trace=False)
    finally:
        if prev is None:
            os.environ.pop("BASS_NEVER_TRACE", None)
        else:
            os.environ["BASS_NEVER_TRACE"] = prev
    return assemble_output(res.results)


# revision 20
# speedup vs baseline: 1.7751x; 1.0196x over previous
"""DCGRU cell (nn_DCGRUCell) Trainium2 Bass kernel, 8 NeuronCores.

Sharding: node dimension N=4096 split 8 ways (512 rows/core); supports are
fed host-transposed (T = A^T), held resident in SBUF as bf16. Hop-1
diffusion products are computed node-major and AllGathered in two
column-chunks per GCN (A: bc cols 0-511; B: cols 512-1023 + 32 ragged) so
the gathers overlap hop-1/hop-2 compute. Hop-2 products are computed
directly in transposed (feature-major) form. All matmul operands are bf16
(PSUM fp32); the Chebyshev combination x2 = 2*A@x1 - x0 is folded into the
dense W matrices host-side. Moving operands are loaded 4 m-tiles per DMA;
the dense W stage processes batches 4 at a time with the candidate
transposes software-pipelined one iteration behind the matmuls. DMA issue
is split across the two HWDGE rings (sync=loads, scalar=stores).

kernel(**inputs) takes the FULL inputs from reference.setup_inputs() and
returns the FULL [16, 4096, 64] float32 output.
"""
import os
import numpy as np

import concourse.bass as bass
import concourse.mybir as mybir
import concourse.tile as tile
from concourse import bacc
from concourse.bass_utils import run_bass_kernel_spmd

F32 = mybir.dt.float32
BF16 = mybir.dt.bfloat16
AF = mybir.ActivationFunctionType

NCORES = 8
B, N, H, DIN = 16, 4096, 64, 2
C = DIN + H                 # 66 features per batch into each GCN
BC = B * C                  # 1056
NOWN = N // NCORES          # 512 rows per core
NT = NOWN // 128            # 4 n-tiles per core
MT = N // 128               # 32 m-tiles (contraction)
NQ = MT // NT               # 8 rank-blocks of 4 m-tiles
CA = 512                    # chunk A: bc columns 0:512
CB = 544                    # chunk B: bc columns 512:1024 + 32 ragged
MAIN = 1024
RAG = BC - MAIN             # 32 ragged columns
NB = 4                      # batches per W-stage iteration
GROUP = [list(range(NCORES))]

_NC_CACHE = {}


def build_nc():
    nc = bacc.Bacc("TRN2", target_bir_lowering=False, debug=False,
                   num_devices=NCORES)

    d = {}
    d["Ts"] = nc.dram_tensor("Ts", [2, N, NOWN], BF16, kind="ExternalInput")
    d["xs_main"] = nc.dram_tensor("xs_main", [N, MAIN], BF16,
                                  kind="ExternalInput")
    d["xs_rag"] = nc.dram_tensor("xs_rag", [MT, 128, RAG], BF16,
                                 kind="ExternalInput")
    d["xsT_own"] = nc.dram_tensor("xsT_own", [BC, NOWN], BF16,
                                  kind="ExternalInput")
    d["Wg"] = nc.dram_tensor("Wg", [5 * C, 2 * H], BF16, kind="ExternalInput")
    d["bg"] = nc.dram_tensor("bg", [2 * H, 1], F32, kind="ExternalInput")
    d["Wu"] = nc.dram_tensor("Wu", [5 * C, H], BF16, kind="ExternalInput")
    d["bu"] = nc.dram_tensor("bu", [H, 1], F32, kind="ExternalInput")
    d["outT"] = nc.dram_tensor("outT", [B, H, NOWN], F32,
                               kind="ExternalOutput")

    with tile.TileContext(nc) as tc:
        _emit(nc, tc, d)
    nc.compile()
    return nc


def _emit(nc, tc, d):
    import contextlib
    stack = contextlib.ExitStack()
    with stack:
        const = stack.enter_context(tc.tile_pool(name="const", bufs=1))
        sb_ex = stack.enter_context(tc.tile_pool(name="ex", bufs=1))
        sb_mov = stack.enter_context(tc.tile_pool(name="mov", bufs=1))
        sb_sm = stack.enter_context(tc.tile_pool(name="small", bufs=1))
        dram = stack.enter_context(
            tc.tile_pool(name="dram", bufs=1, space="DRAM"))
        psum = stack.enter_context(
            tc.tile_pool(name="psum", bufs=1, space="PSUM"))

        # ---- resident support tiles (loaded staggered in first sweep) ----
        Tch = {}
        for s in range(2):
            for k in range(NQ):
                Tch[(s, k)] = const.tile([128, NT, 512], BF16,
                                         name=f"T{s}_{k}")

        def load_Tch(k):
            for s in range(2):
                ts = d["Ts"].ap()[s].rearrange("(t p) n -> p t n", p=128)
                nc.scalar.dma_start(Tch[(s, k)][:],
                                    ts[:, k * NT:(k + 1) * NT, :])

        load_Tch(0)
        load_Tch(1)

        def T_tile(s, m):
            return Tch[(s, m // NT)][:, m % NT, :]

        ident = const.tile([128, 128], F32)
        nc.gpsimd.memset(ident[:], 0.0)
        nc.gpsimd.affine_select(
            out=ident[:], in_=ident[:],
            compare_op=mybir.AluOpType.not_equal, fill=1.0, base=0,
            pattern=[[-1, 128]], channel_multiplier=1)
        identb = const.tile([128, 128], BF16)
        nc.vector.tensor_copy(identb[:], ident[:])

        # dense-W constants: K-tiled [3, 110, out] (DMAs emitted later, just
        # before the gate loop, to keep the scalar ring clear at startup)
        KT = 110
        wg_t = const.tile([KT, 3, 2 * H], BF16)
        wu_t = const.tile([KT, 3, H], BF16)
        bg_t = const.tile([2 * H, 1], F32)
        bu_t = const.tile([H, 1], F32)

        def load_w_consts():
            for k in range(3):
                nc.scalar.dma_start(wg_t[:, k, :],
                                    d["Wg"].ap()[k * KT:(k + 1) * KT, :])
                nc.scalar.dma_start(wu_t[:, k, :],
                                    d["Wu"].ap()[k * KT:(k + 1) * KT, :])
            nc.scalar.dma_start(bg_t[:], d["bg"].ap())
            nc.scalar.dma_start(bu_t[:], d["bu"].ap())

        # row-run map: K-tile k's partition range [off, off+w) reads block
        # j (0 = direct input, 1-4 = diffusion outputs), feature cols
        # [c, c+w)
        KT_RUNS = []
        r0 = 0
        while r0 < 330:
            k, off = divmod(r0, KT)
            j, c = divmod(r0, C)
            w = min(C - c, KT - off)
            KT_RUNS.append((k, off, j, c, w))
            r0 += w

        # ---- DRAM staging ----
        agA = [dram.tile([2 * NT * 128 * CA], BF16, name=f"agA{g}")
               for g in range(2)]
        agB = [dram.tile([2 * NT * 128 * CB], BF16, name=f"agB{g}")
               for g in range(2)]
        agAo = [dram.tile([NCORES * 2 * NT * 128 * CA], BF16,
                          name=f"agAo{g}", addr_space="Shared")
                for g in range(2)]
        agBo = [dram.tile([NCORES * 2 * NT * 128 * CB], BF16,
                          name=f"agBo{g}", addr_space="Shared")
                for g in range(2)]
        candA = dram.tile([NT * 128 * CA], BF16, name="candA")
        candB = dram.tile([NT * 128 * CB], BF16, name="candB")
        candAo = dram.tile([NCORES * NT * 128 * CA], BF16, name="candAo",
                           addr_space="Shared")
        candBo = dram.tile([NCORES * NT * 128 * CB], BF16, name="candBo",
                           addr_space="Shared")
        yt = [dram.tile([BC, NOWN], BF16, name=f"yt{i}") for i in range(4)]
        yt2 = [dram.tile([BC, NOWN], BF16, name=f"yt2_{i}") for i in range(4)]
        candT_dram = dram.tile([BC, NOWN], BF16)
        rt_dram = dram.tile([B, H, NOWN], BF16)

        def agA_own(g, s, t):
            o = ((s * NT + t) * 128) * CA
            return agA[g].opt()[o:o + 128 * CA].rearrange(
                "(p f) -> p f", f=CA)

        def agB_own(g, s, t):
            o = ((s * NT + t) * 128) * CB
            return agB[g].opt()[o:o + 128 * CB].rearrange(
                "(p f) -> p f", f=CB)

        def outA_q(g, s, q):
            # rank q's 4 m-tiles for support s: [p, t, f]
            o = ((q * 2 + s) * NT * 128) * CA
            return agAo[g].opt()[o:o + NT * 128 * CA].rearrange(
                "(t p f) -> p t f", p=128, f=CA)

        def outB_q(g, s, q):
            o = ((q * 2 + s) * NT * 128) * CB
            return agBo[g].opt()[o:o + NT * 128 * CB].rearrange(
                "(t p f) -> p t f", p=128, f=CB)

        def candA_own(t):
            o = t * 128 * CA
            return candA.opt()[o:o + 128 * CA].rearrange(
                "(p f) -> p f", f=CA)

        def candB_own(t):
            o = t * 128 * CB
            return candB.opt()[o:o + 128 * CB].rearrange(
                "(p f) -> p f", f=CB)

        def candAo_q(q):
            o = q * NT * 128 * CA
            return candAo.opt()[o:o + NT * 128 * CA].rearrange(
                "(t p f) -> p t f", p=128, f=CA)

        def candBo_q(q):
            o = q * NT * 128 * CB
            return candBo.opt()[o:o + NT * 128 * CB].rearrange(
                "(t p f) -> p t f", p=128, f=CB)

        def allgather(src, dst):
            nc.gpsimd.collective_compute(
                "AllGather", mybir.AluOpType.bypass, replica_groups=GROUP,
                ins=[src.opt()], outs=[dst.opt()])

        # ============ hop-1: node-major chunks + overlapped AG ============
        def emit_hop1_pair(pid, g, mov_main, mov_rag, yt_dst, stagger_T,
                           ld):
            """Y1_s[own rows, :] = A_s @ M for s in (0, 1); AG chunk A
            issued between the two main sweeps, chunk B after the ragged
            pass. Feature-major yt transposes are deferred (returned as a
            closure) so they land on the PE during the AG-B transfer.
            `ld` is the engine issuing moving-operand loads: for GCN1 the
            scalar ring (loads are AG-independent and must not sit behind
            hop-2's AG-gated loads on the sync ring); for GCN2 the sync
            ring (its loads are AG-gated like everything behind them)."""
            kept = {}
            preloaded = {}
            for hh in range(2):
                ps_m = {}
                for s in range(2):
                    for n in range(NT):
                        ps_m[(s, n)] = psum.tile(
                            [128, 512], F32, name=f"psm{pid}_{hh}{s}{n}",
                            tag="acc", bufs=8)
                for q in range(NQ):
                    if stagger_T and hh == 0 and q + 2 < NQ:
                        load_Tch(q + 2)
                    if (hh, q) in preloaded:
                        mv4 = preloaded.pop((hh, q))
                    else:
                        mv4 = sb_mov.tile([128, NT, 512], BF16,
                                          name=f"mv{pid}_{hh}_{q}",
                                          tag="mov", bufs=4)
                        ld.dma_start(mv4[:], mov_main(q, hh))
                    for tt in range(NT):
                        m = q * NT + tt
                        for s in range(2):
                            for n in range(NT):
                                nc.tensor.matmul(
                                    ps_m[(s, n)][:],
                                    T_tile(s, m)[:, n * 128:(n + 1) * 128],
                                    mv4[:, tt, :], start=(m == 0),
                                    stop=(m == MT - 1))
                if hh == 0:
                    # prefetch the next sweep's first tiles ahead of the
                    # staging stores so the ring never idles the PE at the
                    # sweep boundary
                    for qq in range(2):
                        mv4p = sb_mov.tile([128, NT, 512], BF16,
                                           name=f"mv{pid}_1_{qq}",
                                           tag="mov", bufs=4)
                        ld.dma_start(mv4p[:], mov_main(qq, 1))
                        preloaded[(1, qq)] = mv4p
                else:
                    # issue every ragged-pass load before the h1 staging
                    # stores: the loads must not queue up behind stores
                    # whose data is not ready yet (ring FIFO + shared
                    # completion-semaphore lanes both stall the rag MMs
                    # otherwise)
                    for q in range(NQ):
                        mvr4 = sb_mov.tile([128, NT, RAG], BF16,
                                           name=f"mvr{pid}_{q}",
                                           tag="movr", bufs=8)
                        ld.dma_start(mvr4[:], mov_rag(q))
                        preloaded[("r", q)] = mvr4
                for s in range(2):
                    exhs = []
                    for n in range(NT):
                        exh = sb_ex.tile([128, 512], BF16,
                                         name=f"ex{pid}{hh}{s}{n}",
                                         tag="ex", bufs=16)
                        nc.vector.tensor_copy(exh[:], ps_m[(s, n)][:])
                        dst = (agA_own(g, s, n) if hh == 0
                               else agB_own(g, s, n)[:, 0:512])
                        nc.scalar.dma_start(dst, exh[:])
                        exhs.append(exh)
                    kept[(hh, s)] = exhs
                if hh == 0:
                    allgather(agA[g], agAo[g])

            # ragged pass (node-major staging feeds chunk B)
            ps_t = [psum.tile([RAG, NOWN], F32, name=f"pst{pid}{s}",
                              tag="acc", bufs=8) for s in range(2)]
            for q in range(NQ):
                mvr4 = preloaded.pop(("r", q))
                for tt in range(NT):
                    m = q * NT + tt
                    for s in range(2):
                        nc.tensor.matmul(ps_t[s][:], mvr4[:, tt, :],
                                         T_tile(s, m), start=(m == 0),
                                         stop=(m == MT - 1))
            for s in range(2):
                rag_ex = sb_ex.tile([RAG, NOWN], BF16, name=f"rgex{pid}{s}",
                                    tag="ragex", bufs=2)
                nc.vector.tensor_copy(rag_ex[:], ps_t[s][:])
                nc.scalar.dma_start(yt_dst[s].opt()[MAIN:BC, :], rag_ex[:])
                for t in range(NT):
                    tp = psum.tile([128, RAG], BF16, name=f"rtp{pid}{s}",
                                   tag="acc", bufs=8)
                    nc.tensor.transpose(
                        tp[:], rag_ex[:, t * 128:(t + 1) * 128],
                        identb[0:RAG, 0:RAG])
                    rnm = sb_sm.tile([128, RAG], BF16, name=f"rnm{pid}{s}",
                                     tag="rnm", bufs=2)
                    nc.vector.tensor_copy(rnm[:], tp[:])
                    nc.scalar.dma_start(agB_own(g, s, t)[:, 512:544],
                                        rnm[:])
            allgather(agB[g], agBo[g])

            def deferred_yt():
                for s in range(2):
                    for hh in range(2):
                        for j in range(4):
                            st4 = sb_sm.tile([128, NOWN], BF16,
                                             name=f"st4{pid}", tag="st",
                                             bufs=3)
                            for n in range(NT):
                                tp = psum.tile([128, 128], BF16,
                                               name=f"tp{pid}", tag="acc",
                                               bufs=8)
                                nc.tensor.transpose(
                                    tp[:],
                                    kept[(hh, s)][n][:,
                                                     j * 128:(j + 1) * 128],
                                    identb[:])
                                nc.vector.tensor_copy(
                                    st4[:, n * 128:(n + 1) * 128], tp[:])
                            jj = hh * 4 + j
                            nc.scalar.dma_start(
                                yt_dst[s].opt()[jj * 128:(jj + 1) * 128, :],
                                st4[:])
            return deferred_yt

        # ======= hop-2 product: transposed form (feature-major out) =======
        def emit_hop2_pass(pid, s, g, part, yt_dst):
            """Y2raw^T[bc, own n] = (A_s @ Y1)^T for one column chunk."""
            wid = CA if part == 0 else CB
            ps = [psum.tile([128, NOWN], F32, name=f"ph2{pid}_{j}",
                            tag="acc", bufs=8) for j in range(4)]
            ps_r = None
            if part == 1:
                ps_r = psum.tile([RAG, NOWN], F32, name=f"ph2r{pid}",
                                 tag="acc", bufs=8)
            for q in range(NQ):
                mr4 = sb_mov.tile([128, NT, wid], BF16,
                                  name=f"mr{pid}_{q}",
                                  tag=("mov" if part == 0 else "movB"),
                                  bufs=(4 if part == 0 else 2))
                nc.sync.dma_start(
                    mr4[:], outA_q(g, s, q) if part == 0
                    else outB_q(g, s, q))
                for tt in range(NT):
                    m = q * NT + tt
                    for j in range(4):
                        nc.tensor.matmul(
                            ps[j][:],
                            mr4[:, tt, j * 128:(j + 1) * 128],
                            T_tile(s, m), start=(m == 0),
                            stop=(m == MT - 1))
                    if part == 1:
                        nc.tensor.matmul(ps_r[:], mr4[:, tt, 512:544],
                                         T_tile(s, m), start=(m == 0),
                                         stop=(m == MT - 1))
            for j in range(4):
                exh = sb_ex.tile([128, NOWN], BF16, name=f"h2ex{pid}_{j}",
                                 tag="ex", bufs=16)
                nc.vector.tensor_copy(exh[:], ps[j][:])
                jj = j if part == 0 else 4 + j
                nc.scalar.dma_start(
                    yt_dst.opt()[jj * 128:(jj + 1) * 128, :], exh[:])
            if part == 1:
                exr = sb_ex.tile([RAG, NOWN], BF16, name=f"h2exr{pid}",
                                 tag="ragex", bufs=2)
                nc.vector.tensor_copy(exr[:], ps_r[:])
                nc.scalar.dma_start(yt_dst.opt()[MAIN:BC, :], exr[:])

        # ======================= GCN 1 (gate) =======================
        def g1_main(q, hh):
            src = d["xs_main"].ap().rearrange("(q t p) f -> q p t f",
                                              p=128, t=NT)
            return src[q, :, :, hh * 512:(hh + 1) * 512]

        def g1_rag(q):
            src = d["xs_rag"].ap().rearrange("(q t) p f -> q p t f", t=NT)
            return src[q]

        dyt1 = emit_hop1_pair("g1h1", 0, g1_main, g1_rag,
                              (yt[0], yt[2]), True, nc.scalar)
        dyt1()
        emit_hop2_pass("g1s0h2A", 0, 0, 0, yt[1])
        emit_hop2_pass("g1s1h2A", 1, 0, 0, yt[3])
        emit_hop2_pass("g1s0h2B", 0, 0, 1, yt[1])
        emit_hop2_pass("g1s1h2B", 1, 0, 1, yt[3])

        # ----- gate W-stage + candidate build, 4 batches per iter; the
        # candidate transpose/staging runs one iteration behind so the PE
        # queue alternates matmul-block, transpose-block without stalling.
        def load_ktiles(pi, block0_src, ysrc):
            """Assemble the 330-feature contraction as 3 dense K-tiles of
            110 partitions; block 0 comes from block0_src, blocks 1-4 from
            ysrc[j-1] (the staged diffusion outputs)."""
            b0 = NB * pi
            kts = [sb_sm.tile([KT, NB, NOWN], BF16, name=f"kt{k}",
                              tag=f"kt{k}", bufs=2) for k in range(3)]
            for k, off, j, c, w in KT_RUNS:
                src = block0_src if j == 0 else ysrc[j - 1].opt()
                nc.sync.dma_start(
                    kts[k][off:off + w, :, :],
                    src[b0 * C:(b0 + NB) * C, :]
                    .rearrange("(b c) n -> c b n", b=NB)[c:c + w])
            return kts

        def gate_mm(pi):
            b0 = NB * pi
            kts = load_ktiles(pi, d["xsT_own"].ap(), yt)
            zr_ps = [psum.tile([2 * H, NOWN], F32, name=f"zrps{b2}",
                               tag="acc", bufs=8) for b2 in range(NB)]
            for k in range(3):
                for b2 in range(NB):
                    nc.tensor.matmul(zr_ps[b2][:], wg_t[:, k, :],
                                     kts[k][:, b2, :],
                                     start=(k == 0), stop=(k == 2))
            zr4 = sb_sm.tile([2 * H, NB, NOWN], BF16, name="zr", tag="zr",
                             bufs=2)
            for b2 in range(NB):
                nc.scalar.activation(zr4[:, b2, :], zr_ps[b2][:],
                                     AF.Sigmoid, bias=bg_t[:])
            nc.scalar.dma_start(
                rt_dram.opt()[b0:b0 + NB].rearrange("b (h n) -> h b n",
                                                    h=H),
                zr4[H:2 * H, :, :])
            # kts[0][0:C] holds the direct input rows [state(64); x(2)]
            cT4 = sb_sm.tile([C, NB, NOWN], BF16, name="cT", tag="cT",
                             bufs=2)
            nc.vector.tensor_mul(cT4[0:H, :, :], zr4[0:H, :, :],
                                 kts[0][0:H, :, :])
            nc.vector.tensor_copy(cT4[H:C, :, :], kts[0][H:C, :, :])
            nc.scalar.dma_start(
                candT_dram.opt()[b0 * C:(b0 + NB) * C, :]
                .rearrange("(b c) n -> c b n", b=NB), cT4[:])
            return cT4

        def cand_stage(pi, cT4):
            c0 = NB * pi * C
            hi = c0 + NB * C
            for t in range(NT):
                ct4 = sb_sm.tile([128, NB, C], BF16, name="ctnm",
                                 tag="ctnm", bufs=2)
                for b2 in range(NB):
                    tp = psum.tile([128, C], BF16, name="ctps", tag="acc",
                                   bufs=8)
                    nc.tensor.transpose(
                        tp[:], cT4[:, b2, t * 128:(t + 1) * 128],
                        identb[0:C, 0:C])
                    nc.vector.tensor_copy(ct4[:, b2, :], tp[:])
                flat = ct4[:].rearrange("p b c -> p (b c)")
                segs = []
                if c0 < CA:
                    e = min(hi, CA)
                    segs.append((candA_own(t)[:, c0:e], 0, e - c0))
                if hi > CA and c0 < MAIN:
                    s0 = max(c0, CA)
                    e = min(hi, MAIN)
                    segs.append((candB_own(t)[:, s0 - CA:e - CA],
                                 s0 - c0, e - s0))
                if hi > MAIN:
                    s0 = max(c0, MAIN)
                    segs.append((candB_own(t)[:, 512 + s0 - MAIN:
                                              512 + hi - MAIN],
                                 s0 - c0, hi - s0))
                for dst, off, w in segs:
                    nc.scalar.dma_start(dst, flat[:, off:off + w])

        load_w_consts()
        prev = None
        for pi in range(B // NB):
            cur = gate_mm(pi)
            if prev is not None:
                cand_stage(pi - 1, prev)
                if pi - 1 == 1:
                    allgather(candA, candAo)
            prev = cur
        cand_stage(B // NB - 1, prev)
        allgather(candB, candBo)

        # ======================= GCN 2 (update) =======================
        def g2_main(q, hh):
            if hh == 0:
                return candAo_q(q)
            return candBo_q(q)[:, :, 0:512]

        def g2_rag(q):
            return candBo_q(q)[:, :, 512:544]

        dyt2 = emit_hop1_pair("g2h1", 1, g2_main, g2_rag,
                              (yt2[0], yt2[2]), False, nc.sync)
        dyt2()
        emit_hop2_pass("g2s0h2A", 0, 1, 0, yt2[1])
        emit_hop2_pass("g2s1h2A", 1, 1, 0, yt2[3])
        emit_hop2_pass("g2s0h2B", 0, 1, 1, yt2[1])
        emit_hop2_pass("g2s1h2B", 1, 1, 1, yt2[3])

        # update W-stage + final combine, 4 batches per iter
        for pi in range(B // NB):
            b0 = NB * pi
            kts = load_ktiles(pi, candT_dram.opt(), yt2)
            hc_ps = [psum.tile([H, NOWN], F32, name=f"hcps{b2}", tag="acc",
                               bufs=8) for b2 in range(NB)]
            for k in range(3):
                for b2 in range(NB):
                    nc.tensor.matmul(hc_ps[b2][:], wu_t[:, k, :],
                                     kts[k][:, b2, :],
                                     start=(k == 0), stop=(k == 2))
            hc4 = sb_sm.tile([H, NB, NOWN], BF16, name="hc", tag="zr",
                             bufs=2)
            for b2 in range(NB):
                nc.scalar.activation(hc4[:, b2, :], hc_ps[b2][:], AF.Tanh,
                                     bias=bu_t[:])

            # out = hc + r * (state - hc); state rows are xsT_own[0:H]
            stT4 = sb_sm.tile([H, NB, NOWN], BF16, name="stTu", tag="stg",
                              bufs=2)
            nc.sync.dma_start(
                stT4[:],
                d["xsT_own"].ap()[b0 * C:(b0 + NB) * C, :]
                .rearrange("(b c) n -> c b n", b=NB)[0:H])
            rT4 = sb_sm.tile([H, NB, NOWN], BF16, name="rT", tag="rT",
                             bufs=2)
            nc.sync.dma_start(
                rT4[:],
                rt_dram.opt()[b0:b0 + NB].rearrange("b (h n) -> h b n",
                                                    h=H))
            tmp4 = sb_sm.tile([H, NB, NOWN], BF16, name="tmp", tag="tmp",
                              bufs=2)
            nc.vector.tensor_sub(tmp4[:], stT4[:], hc4[:])
            nc.vector.tensor_mul(tmp4[:], rT4[:], tmp4[:])
            ot4 = sb_sm.tile([H, NB, NOWN], F32, name="ot", tag="ot",
                             bufs=1)
            nc.vector.tensor_add(ot4[:], hc4[:], tmp4[:])
            nc.scalar.dma_start(
                d["outT"].ap()[b0:b0 + NB].rearrange("b h n -> h b n"),
                ot4[:])


def prepare_in_maps(x, state, support0, support1, W_gate, b_gate,
                    W_update, b_update):
    BFNP = mybir.dt.np(BF16)
    xs = np.concatenate([x, state], axis=-1)          # [B, N, C]
    xs_nm = np.ascontiguousarray(
        xs.transpose(1, 0, 2).reshape(N, BC)).astype(BFNP)
    # feature-major input for W / elementwise uses [state(64); x(2)] rows
    sx_nm = np.ascontiguousarray(
        np.concatenate([state, x], axis=-1)
        .transpose(1, 0, 2).reshape(N, BC)).astype(np.float32)
    perm = np.r_[DIN:C, 0:DIN]                 # [x, state] -> [state, x]

    # fold the Chebyshev combination x2 = 2*A@x1 - x0 into W:
    # W0 -= (W2 + W4); W2 *= 2; W4 *= 2  (per 66-row block)
    def fold(W):
        Wf = np.ascontiguousarray(W, dtype=np.float32).copy()
        Wf[0:C] -= Wf[2 * C:3 * C] + Wf[4 * C:5 * C]
        Wf[2 * C:3 * C] *= 2.0
        Wf[4 * C:5 * C] *= 2.0
        return Wf

    Wg_dev = fold(W_gate)
    Wg_dev[0:C] = Wg_dev[0:C][perm]            # only the X-block reads xsT
    Wu_dev = fold(W_update)
    for j in range(5):                         # all of cand's blocks permute
        Wu_dev[j * C:(j + 1) * C] = Wu_dev[j * C:(j + 1) * C][perm]
    Wg_dev = Wg_dev.astype(BFNP)
    Wu_dev = Wu_dev.astype(BFNP)

    xs_main = np.ascontiguousarray(xs_nm[:, :MAIN])
    xs_rag = np.ascontiguousarray(xs_nm[:, MAIN:]).reshape(MT, 128, RAG)
    bg = np.ascontiguousarray(b_gate, dtype=np.float32).reshape(2 * H, 1)
    bu = np.ascontiguousarray(b_update, dtype=np.float32).reshape(H, 1)
    s0b = np.asarray(support0, dtype=np.float32).astype(BFNP)
    s1b = np.asarray(support1, dtype=np.float32).astype(BFNP)

    in_maps = []
    for r in range(NCORES):
        n0 = r * NOWN
        in_maps.append({
            "Ts": np.ascontiguousarray(
                np.stack([s0b[n0:n0 + NOWN, :].T,
                          s1b[n0:n0 + NOWN, :].T])),
            "xs_main": xs_main,
            "xs_rag": xs_rag,
            "xsT_own": np.ascontiguousarray(
                sx_nm[n0:n0 + NOWN].T).astype(BFNP),
            "Wg": Wg_dev, "bg": bg, "Wu": Wu_dev, "bu": bu,
        })
    return in_maps


def assemble_output(results):
    out = np.empty((B, N, H), dtype=np.float32)
    for r in range(NCORES):
        n0 = r * NOWN
        out[:, n0:n0 + NOWN, :] = results[r]["outT"].transpose(0, 2, 1)
    return out


def get_nc():
    if "nc" not in _NC_CACHE:
        _NC_CACHE["nc"] = build_nc()
    return _NC_CACHE["nc"]


def kernel(x, state, support0, support1, W_gate, b_gate, W_update, b_update):
    nc = get_nc()
    in_maps = prepare_in_maps(x, state, support0, support1,
                              W_gate, b_gate, W_update, b_update)
    prev = os.environ.get("BASS_NEVER_TRACE")
    os.environ["BASS_NEVER_TRACE"] = "1"
    try:
        res = run_bass_kernel_spmd(nc, in_maps, list(range(NCORES)),
                                   trace=False)
    finally:
        if prev is None:
            os.environ.pop("BASS_NEVER_TRACE", None)
        else:
            os.environ["BASS_NEVER_TRACE"] = prev
    return assemble_output(res.results)


# revision 25
# speedup vs baseline: 1.8919x; 1.0658x over previous
"""DCGRU cell (nn_DCGRUCell) Trainium2 Bass kernel, 8 NeuronCores.

Sharding: node dimension N=4096 split 8 ways (512 rows/core); supports are
fed host-transposed (T = A^T), held resident in SBUF as bf16. Hop-1
diffusion products are computed node-major and AllGathered in two
column-chunks per GCN (A: bc cols 0-511; B: cols 512-1023 + 32 ragged) so
the gathers overlap hop-1/hop-2 compute. Hop-2 products are computed
directly in transposed (feature-major) form. All matmul operands are bf16
(PSUM fp32); the Chebyshev combination x2 = 2*A@x1 - x0 is folded into the
dense W matrices host-side. Moving operands are loaded 4 m-tiles per DMA;
the dense W stage processes batches 4 at a time with the candidate
transposes software-pipelined one iteration behind the matmuls. DMA issue
is split across the two HWDGE rings (sync=loads, scalar=stores).

kernel(**inputs) takes the FULL inputs from reference.setup_inputs() and
returns the FULL [16, 4096, 64] float32 output.
"""
import os
import numpy as np

import concourse.bass as bass
import concourse.mybir as mybir
import concourse.tile as tile
from concourse import bacc
from concourse.bass_utils import run_bass_kernel_spmd

F32 = mybir.dt.float32
BF16 = mybir.dt.bfloat16
AF = mybir.ActivationFunctionType

NCORES = 8
B, N, H, DIN = 16, 4096, 64, 2
C = DIN + H                 # 66 features per batch into each GCN
BC = B * C                  # 1056
NOWN = N // NCORES          # 512 rows per core
NT = NOWN // 128            # 4 n-tiles per core
MT = N // 128               # 32 m-tiles (contraction)
NQ = MT // NT               # 8 rank-blocks of 4 m-tiles
CA = 512                    # chunk A: bc columns 0:512
CB = 544                    # chunk B: bc columns 512:1024 + 32 ragged
MAIN = 1024
RAG = BC - MAIN             # 32 ragged columns
NB = 4                      # batches per W-stage iteration
GROUP = [list(range(NCORES))]

_NC_CACHE = {}


def build_nc():
    nc = bacc.Bacc("TRN2", target_bir_lowering=False, debug=False,
                   num_devices=NCORES)

    d = {}
    d["Ts"] = nc.dram_tensor("Ts", [2, N, NOWN], BF16, kind="ExternalInput")
    d["xs_main"] = nc.dram_tensor("xs_main", [N, MAIN], BF16,
                                  kind="ExternalInput")
    d["xs_rag"] = nc.dram_tensor("xs_rag", [MT, 128, RAG], BF16,
                                 kind="ExternalInput")
    d["xsT_own"] = nc.dram_tensor("xsT_own", [BC, NOWN], BF16,
                                  kind="ExternalInput")
    d["Wg"] = nc.dram_tensor("Wg", [5 * C, 2 * H], BF16, kind="ExternalInput")
    d["bg"] = nc.dram_tensor("bg", [2 * H, 1], F32, kind="ExternalInput")
    d["Wu"] = nc.dram_tensor("Wu", [5 * C, H], BF16, kind="ExternalInput")
    d["bu"] = nc.dram_tensor("bu", [H, 1], F32, kind="ExternalInput")
    d["outT"] = nc.dram_tensor("outT", [B, H, NOWN], F32,
                               kind="ExternalOutput")

    with tile.TileContext(nc) as tc:
        _emit(nc, tc, d)
    nc.compile()
    return nc


def _emit(nc, tc, d):
    import contextlib
    stack = contextlib.ExitStack()
    with stack:
        const = stack.enter_context(tc.tile_pool(name="const", bufs=1))
        sb_ex = stack.enter_context(tc.tile_pool(name="ex", bufs=1))
        sb_mov = stack.enter_context(tc.tile_pool(name="mov", bufs=1))
        sb_sm = stack.enter_context(tc.tile_pool(name="small", bufs=1))
        dram = stack.enter_context(
            tc.tile_pool(name="dram", bufs=1, space="DRAM"))
        psum = stack.enter_context(
            tc.tile_pool(name="psum", bufs=1, space="PSUM"))

        # ---- resident support tiles (loaded staggered in first sweep) ----
        Tch = {}
        for s in range(2):
            for k in range(NQ):
                Tch[(s, k)] = const.tile([128, NT, 512], BF16,
                                         name=f"T{s}_{k}")

        def load_Tch(k):
            for s in range(2):
                ts = d["Ts"].ap()[s].rearrange("(t p) n -> p t n", p=128)
                nc.scalar.dma_start(Tch[(s, k)][:],
                                    ts[:, k * NT:(k + 1) * NT, :])

        load_Tch(0)
        load_Tch(1)

        def T_tile(s, m):
            return Tch[(s, m // NT)][:, m % NT, :]

        ident = const.tile([128, 128], F32)
        nc.gpsimd.memset(ident[:], 0.0)
        nc.gpsimd.affine_select(
            out=ident[:], in_=ident[:],
            compare_op=mybir.AluOpType.not_equal, fill=1.0, base=0,
            pattern=[[-1, 128]], channel_multiplier=1)
        identb = const.tile([128, 128], BF16)
        nc.vector.tensor_copy(identb[:], ident[:])

        # dense-W constants: K-tiled [3, 110, out] (DMAs emitted later, just
        # before the gate loop, to keep the scalar ring clear at startup)
        KT = 110
        wg_t = const.tile([KT, 3, 2 * H], BF16)
        wu_t = const.tile([KT, 3, H], BF16)
        bg_t = const.tile([2 * H, 1], F32)
        bu_t = const.tile([H, 1], F32)

        def load_w_consts():
            for k in range(3):
                nc.scalar.dma_start(wg_t[:, k, :],
                                    d["Wg"].ap()[k * KT:(k + 1) * KT, :])
                nc.scalar.dma_start(wu_t[:, k, :],
                                    d["Wu"].ap()[k * KT:(k + 1) * KT, :])
            nc.scalar.dma_start(bg_t[:], d["bg"].ap())
            nc.scalar.dma_start(bu_t[:], d["bu"].ap())

        # row-run map: K-tile k's partition range [off, off+w) reads block
        # j (0 = direct input, 1-4 = diffusion outputs), feature cols
        # [c, c+w)
        KT_RUNS = []
        r0 = 0
        while r0 < 330:
            k, off = divmod(r0, KT)
            j, c = divmod(r0, C)
            w = min(C - c, KT - off)
            KT_RUNS.append((k, off, j, c, w))
            r0 += w

        # ---- DRAM staging ----
        agA = [dram.tile([2 * NT * 128 * CA], BF16, name=f"agA{g}")
               for g in range(2)]
        agB = [dram.tile([2 * NT * 128 * 512], BF16, name=f"agB{g}")
               for g in range(2)]
        agR = [dram.tile([2 * NT * 128 * RAG], BF16, name=f"agR{g}")
               for g in range(2)]
        agAo = [dram.tile([NCORES * 2 * NT * 128 * CA], BF16,
                          name=f"agAo{g}", addr_space="Shared")
                for g in range(2)]
        agBo = [dram.tile([NCORES * 2 * NT * 128 * 512], BF16,
                          name=f"agBo{g}", addr_space="Shared")
                for g in range(2)]
        agRo = [dram.tile([NCORES * 2 * NT * 128 * RAG], BF16,
                          name=f"agRo{g}", addr_space="Shared")
                for g in range(2)]
        candA = dram.tile([NT * 128 * CA], BF16, name="candA")
        candB = dram.tile([NT * 128 * 512], BF16, name="candB")
        candR = dram.tile([NT * 128 * RAG], BF16, name="candR")
        candAo = dram.tile([NCORES * NT * 128 * CA], BF16, name="candAo",
                           addr_space="Shared")
        candBo = dram.tile([NCORES * NT * 128 * 512], BF16, name="candBo",
                           addr_space="Shared")
        candRo = dram.tile([NCORES * NT * 128 * RAG], BF16, name="candRo",
                           addr_space="Shared")
        yt = [dram.tile([BC, NOWN], BF16, name=f"yt{i}") for i in range(4)]
        yt2 = [dram.tile([BC, NOWN], BF16, name=f"yt2_{i}") for i in range(4)]
        candT_dram = dram.tile([BC, NOWN], BF16)
        rt_dram = dram.tile([B, H, NOWN], BF16)

        def agA_own(g, s, t):
            o = ((s * NT + t) * 128) * CA
            return agA[g].opt()[o:o + 128 * CA].rearrange(
                "(p f) -> p f", f=CA)

        def agB_own(g, s, t):
            o = ((s * NT + t) * 128) * 512
            return agB[g].opt()[o:o + 128 * 512].rearrange(
                "(p f) -> p f", f=512)

        def agR_own(g, s, t):
            o = ((s * NT + t) * 128) * RAG
            return agR[g].opt()[o:o + 128 * RAG].rearrange(
                "(p f) -> p f", f=RAG)

        def outA_q(g, s, q):
            # rank q's 4 m-tiles for support s: [p, t, f]
            o = ((q * 2 + s) * NT * 128) * CA
            return agAo[g].opt()[o:o + NT * 128 * CA].rearrange(
                "(t p f) -> p t f", p=128, f=CA)

        def outB_q(g, s, q):
            o = ((q * 2 + s) * NT * 128) * 512
            return agBo[g].opt()[o:o + NT * 128 * 512].rearrange(
                "(t p f) -> p t f", p=128, f=512)

        def outR_s(g, s):
            # all ranks' rag blocks for support s: [p, q, t, f]
            v = agRo[g].opt().rearrange("(q s2 t p f) -> s2 p q t f",
                                        s2=2, t=NT, p=128, f=RAG)
            return v[s]

        def candA_own(t):
            o = t * 128 * CA
            return candA.opt()[o:o + 128 * CA].rearrange(
                "(p f) -> p f", f=CA)

        def candB_own(t):
            o = t * 128 * 512
            return candB.opt()[o:o + 128 * 512].rearrange(
                "(p f) -> p f", f=512)

        def candR_own(t):
            o = t * 128 * RAG
            return candR.opt()[o:o + 128 * RAG].rearrange(
                "(p f) -> p f", f=RAG)

        def candAo_q(q):
            o = q * NT * 128 * CA
            return candAo.opt()[o:o + NT * 128 * CA].rearrange(
                "(t p f) -> p t f", p=128, f=CA)

        def candBo_q(q):
            o = q * NT * 128 * 512
            return candBo.opt()[o:o + NT * 128 * 512].rearrange(
                "(t p f) -> p t f", p=128, f=512)

        def candRo_q(q):
            o = q * NT * 128 * RAG
            return candRo.opt()[o:o + NT * 128 * RAG].rearrange(
                "(t p f) -> p t f", p=128, f=RAG)

        def allgather(src, dst):
            nc.gpsimd.collective_compute(
                "AllGather", mybir.AluOpType.bypass, replica_groups=GROUP,
                ins=[src.opt()], outs=[dst.opt()])

        # ============ hop-1: node-major chunks + overlapped AG ============
        def emit_hop1_pair(pid, g, mov_main, mov_rag, yt_dst, stagger_T,
                           ld):
            """Y1_s[own rows, :] = A_s @ M for s in (0, 1); AG chunk A
            issued between the two main sweeps, chunk B after the ragged
            pass. Feature-major yt transposes are deferred (returned as a
            closure) so they land on the PE during the AG-B transfer.
            `ld` is the engine issuing moving-operand loads: for GCN1 the
            scalar ring (loads are AG-independent and must not sit behind
            hop-2's AG-gated loads on the sync ring); for GCN2 the sync
            ring (its loads are AG-gated like everything behind them)."""
            kept = {}
            preloaded = {}
            for hh in range(2):
                ps_m = {}
                for s in range(2):
                    for n in range(NT):
                        ps_m[(s, n)] = psum.tile(
                            [128, 512], F32, name=f"psm{pid}_{hh}{s}{n}",
                            tag="acc", bufs=8)
                for q in range(NQ):
                    if stagger_T and hh == 0 and q + 2 < NQ:
                        load_Tch(q + 2)
                    if (hh, q) in preloaded:
                        mv4 = preloaded.pop((hh, q))
                    else:
                        mv4 = sb_mov.tile([128, NT, 512], BF16,
                                          name=f"mv{pid}_{hh}_{q}",
                                          tag="mov", bufs=4)
                        ld.dma_start(mv4[:], mov_main(q, hh))
                    for tt in range(NT):
                        m = q * NT + tt
                        for s in range(2):
                            for n in range(NT):
                                nc.tensor.matmul(
                                    ps_m[(s, n)][:],
                                    T_tile(s, m)[:, n * 128:(n + 1) * 128],
                                    mv4[:, tt, :], start=(m == 0),
                                    stop=(m == MT - 1))
                if hh == 0:
                    # prefetch the next sweep's first tiles ahead of the
                    # staging stores so the ring never idles the PE at the
                    # sweep boundary
                    for qq in range(2):
                        mv4p = sb_mov.tile([128, NT, 512], BF16,
                                           name=f"mv{pid}_1_{qq}",
                                           tag="mov", bufs=4)
                        ld.dma_start(mv4p[:], mov_main(qq, 1))
                        preloaded[(1, qq)] = mv4p
                else:
                    # issue every ragged-pass load before the h1 staging
                    # stores: the loads must not queue up behind stores
                    # whose data is not ready yet (ring FIFO + shared
                    # completion-semaphore lanes both stall the rag MMs
                    # otherwise)
                    for q in range(NQ):
                        mvr4 = sb_mov.tile([128, NT, RAG], BF16,
                                           name=f"mvr{pid}_{q}",
                                           tag="movr", bufs=8)
                        ld.dma_start(mvr4[:], mov_rag(q))
                        preloaded[("r", q)] = mvr4
                for s in range(2):
                    exhs = []
                    for n in range(NT):
                        exh = sb_ex.tile([128, 512], BF16,
                                         name=f"ex{pid}{hh}{s}{n}",
                                         tag="ex", bufs=16)
                        nc.vector.tensor_copy(exh[:], ps_m[(s, n)][:])
                        dst = (agA_own(g, s, n) if hh == 0
                               else agB_own(g, s, n))
                        nc.scalar.dma_start(dst, exh[:])
                        exhs.append(exh)
                    kept[(hh, s)] = exhs
                allgather(*((agA[g], agAo[g]) if hh == 0
                            else (agB[g], agBo[g])))

            # ragged pass (node-major staging feeds chunk B)
            ps_t = [psum.tile([RAG, NOWN], F32, name=f"pst{pid}{s}",
                              tag="acc", bufs=8) for s in range(2)]
            for q in range(NQ):
                mvr4 = preloaded.pop(("r", q))
                for tt in range(NT):
                    m = q * NT + tt
                    for s in range(2):
                        nc.tensor.matmul(ps_t[s][:], mvr4[:, tt, :],
                                         T_tile(s, m), start=(m == 0),
                                         stop=(m == MT - 1))
            for s in range(2):
                rag_ex = sb_ex.tile([RAG, NOWN], BF16, name=f"rgex{pid}{s}",
                                    tag="ragex", bufs=2)
                nc.vector.tensor_copy(rag_ex[:], ps_t[s][:])
                nc.scalar.dma_start(yt_dst[s].opt()[MAIN:BC, :], rag_ex[:])
                for t in range(NT):
                    tp = psum.tile([128, RAG], BF16, name=f"rtp{pid}{s}",
                                   tag="acc", bufs=8)
                    nc.tensor.transpose(
                        tp[:], rag_ex[:, t * 128:(t + 1) * 128],
                        identb[0:RAG, 0:RAG])
                    rnm = sb_sm.tile([128, RAG], BF16, name=f"rnm{pid}{s}",
                                     tag="rnm", bufs=2)
                    nc.vector.tensor_copy(rnm[:], tp[:])
                    nc.scalar.dma_start(agR_own(g, s, t), rnm[:])
            allgather(agR[g], agRo[g])

            def deferred_yt():
                for s in range(2):
                    for hh in range(2):
                        for j in range(4):
                            st4 = sb_sm.tile([128, NOWN], BF16,
                                             name=f"st4{pid}", tag="st",
                                             bufs=3)
                            for n in range(NT):
                                tp = psum.tile([128, 128], BF16,
                                               name=f"tp{pid}", tag="acc",
                                               bufs=8)
                                nc.tensor.transpose(
                                    tp[:],
                                    kept[(hh, s)][n][:,
                                                     j * 128:(j + 1) * 128],
                                    identb[:])
                                nc.vector.tensor_copy(
                                    st4[:, n * 128:(n + 1) * 128], tp[:])
                            jj = hh * 4 + j
                            nc.scalar.dma_start(
                                yt_dst[s].opt()[jj * 128:(jj + 1) * 128, :],
                                st4[:])
            return deferred_yt

        # ======= hop-2 product: transposed form (feature-major out) =======
        def emit_hop2_pass(pid, s, g, part, yt_dst):
            """Y2raw^T[bc, own n] = (A_s @ Y1)^T for one column chunk."""
            ps = [psum.tile([128, NOWN], F32, name=f"ph2{pid}_{j}",
                            tag="acc", bufs=8) for j in range(4)]
            ps_r = None
            if part == 1:
                ps_r = psum.tile([RAG, NOWN], F32, name=f"ph2r{pid}",
                                 tag="acc", bufs=8)
                mrR = sb_mov.tile([128, NT, NQ, RAG], BF16,
                                  name=f"mrR{pid}", tag="mrR", bufs=2)
                vR = outR_s(g, s)
                for t in range(NT):
                    nc.sync.dma_start(mrR[:, t, :, :], vR[:, :, t, :])
            for q in range(NQ):
                mr4 = sb_mov.tile([128, NT, 512], BF16,
                                  name=f"mr{pid}_{q}", tag="mov", bufs=4)
                nc.sync.dma_start(
                    mr4[:], outA_q(g, s, q) if part == 0
                    else outB_q(g, s, q))
                for tt in range(NT):
                    m = q * NT + tt
                    for j in range(4):
                        nc.tensor.matmul(
                            ps[j][:],
                            mr4[:, tt, j * 128:(j + 1) * 128],
                            T_tile(s, m), start=(m == 0),
                            stop=(m == MT - 1))
                    if part == 1:
                        nc.tensor.matmul(ps_r[:], mrR[:, tt, q, :],
                                         T_tile(s, m), start=(m == 0),
                                         stop=(m == MT - 1))
            for j in range(4):
                exh = sb_ex.tile([128, NOWN], BF16, name=f"h2ex{pid}_{j}",
                                 tag="ex", bufs=16)
                nc.vector.tensor_copy(exh[:], ps[j][:])
                jj = j if part == 0 else 4 + j
                nc.scalar.dma_start(
                    yt_dst.opt()[jj * 128:(jj + 1) * 128, :], exh[:])
            if part == 1:
                exr = sb_ex.tile([RAG, NOWN], BF16, name=f"h2exr{pid}",
                                 tag="ragex", bufs=2)
                nc.vector.tensor_copy(exr[:], ps_r[:])
                nc.scalar.dma_start(yt_dst.opt()[MAIN:BC, :], exr[:])

        # ======================= GCN 1 (gate) =======================
        def g1_main(q, hh):
            src = d["xs_main"].ap().rearrange("(q t p) f -> q p t f",
                                              p=128, t=NT)
            return src[q, :, :, hh * 512:(hh + 1) * 512]

        def g1_rag(q):
            src = d["xs_rag"].ap().rearrange("(q t) p f -> q p t f", t=NT)
            return src[q]

        dyt1 = emit_hop1_pair("g1h1", 0, g1_main, g1_rag,
                              (yt[0], yt[2]), True, nc.scalar)
        dyt1()
        emit_hop2_pass("g1s0h2A", 0, 0, 0, yt[1])
        emit_hop2_pass("g1s1h2A", 1, 0, 0, yt[3])
        emit_hop2_pass("g1s0h2B", 0, 0, 1, yt[1])
        emit_hop2_pass("g1s1h2B", 1, 0, 1, yt[3])

        # ----- gate W-stage + candidate build, 4 batches per iter; the
        # candidate transpose/staging runs one iteration behind so the PE
        # queue alternates matmul-block, transpose-block without stalling.
        def load_ktiles(pi, block0_src, ysrc):
            """Assemble the 330-feature contraction as 3 dense K-tiles of
            110 partitions; block 0 comes from block0_src, blocks 1-4 from
            ysrc[j-1] (the staged diffusion outputs)."""
            b0 = NB * pi
            kts = [sb_sm.tile([KT, NB, NOWN], BF16, name=f"kt{k}",
                              tag=f"kt{k}", bufs=3) for k in range(3)]
            for k, off, j, c, w in KT_RUNS:
                src = block0_src if j == 0 else ysrc[j - 1].opt()
                nc.sync.dma_start(
                    kts[k][off:off + w, :, :],
                    src[b0 * C:(b0 + NB) * C, :]
                    .rearrange("(b c) n -> c b n", b=NB)[c:c + w])
            return kts

        def gate_mm(pi):
            b0 = NB * pi
            kts = load_ktiles(pi, d["xsT_own"].ap(), yt)
            zr_ps = [psum.tile([2 * H, NOWN], F32, name=f"zrps{b2}",
                               tag="acc", bufs=8) for b2 in range(NB)]
            for k in range(3):
                for b2 in range(NB):
                    nc.tensor.matmul(zr_ps[b2][:], wg_t[:, k, :],
                                     kts[k][:, b2, :],
                                     start=(k == 0), stop=(k == 2))
            zr4 = sb_sm.tile([2 * H, NB, NOWN], BF16, name="zr", tag="zr",
                             bufs=2)
            for b2 in range(NB):
                nc.scalar.activation(zr4[:, b2, :], zr_ps[b2][:],
                                     AF.Sigmoid, bias=bg_t[:])
            nc.scalar.dma_start(
                rt_dram.opt()[b0:b0 + NB].rearrange("b (h n) -> h b n",
                                                    h=H),
                zr4[H:2 * H, :, :])
            # kts[0][0:C] holds the direct input rows [state(64); x(2)]
            cT4 = sb_sm.tile([C, NB, NOWN], BF16, name="cT", tag="cT",
                             bufs=2)
            nc.vector.tensor_mul(cT4[0:H, :, :], zr4[0:H, :, :],
                                 kts[0][0:H, :, :])
            nc.vector.tensor_copy(cT4[H:C, :, :], kts[0][H:C, :, :])
            nc.scalar.dma_start(
                candT_dram.opt()[b0 * C:(b0 + NB) * C, :]
                .rearrange("(b c) n -> c b n", b=NB), cT4[:])
            return cT4

        def cand_stage(pi, cT4):
            c0 = NB * pi * C
            hi = c0 + NB * C
            for t in range(NT):
                ct4 = sb_sm.tile([128, NB, C], BF16, name="ctnm",
                                 tag="ctnm", bufs=2)
                for b2 in range(NB):
                    tp = psum.tile([128, C], BF16, name="ctps", tag="acc",
                                   bufs=8)
                    nc.tensor.transpose(
                        tp[:], cT4[:, b2, t * 128:(t + 1) * 128],
                        identb[0:C, 0:C])
                    nc.vector.tensor_copy(ct4[:, b2, :], tp[:])
                flat = ct4[:].rearrange("p b c -> p (b c)")
                segs = []
                if c0 < CA:
                    e = min(hi, CA)
                    segs.append((candA_own(t)[:, c0:e], 0, e - c0))
                if hi > CA and c0 < MAIN:
                    s0 = max(c0, CA)
                    e = min(hi, MAIN)
                    segs.append((candB_own(t)[:, s0 - CA:e - CA],
                                 s0 - c0, e - s0))
                if hi > MAIN:
                    s0 = max(c0, MAIN)
                    segs.append((candR_own(t)[:, s0 - MAIN:hi - MAIN],
                                 s0 - c0, hi - s0))
                for dst, off, w in segs:
                    nc.scalar.dma_start(dst, flat[:, off:off + w])

        load_w_consts()
        cts = [gate_mm(0), gate_mm(1)]
        cand_stage(0, cts[0])
        cand_stage(1, cts[1])
        allgather(candA, candAo)
        cts.append(gate_mm(2))
        cts.append(gate_mm(3))
        cand_stage(2, cts[2])
        cand_stage(3, cts[3])
        allgather(candB, candBo)
        allgather(candR, candRo)

        # ======================= GCN 2 (update) =======================
        def g2_main(q, hh):
            if hh == 0:
                return candAo_q(q)
            return candBo_q(q)

        def g2_rag(q):
            return candRo_q(q)

        dyt2 = emit_hop1_pair("g2h1", 1, g2_main, g2_rag,
                              (yt2[0], yt2[2]), False, nc.sync)
        dyt2()
        emit_hop2_pass("g2s0h2A", 0, 1, 0, yt2[1])
        emit_hop2_pass("g2s1h2A", 1, 1, 0, yt2[3])
        emit_hop2_pass("g2s0h2B", 0, 1, 1, yt2[1])
        emit_hop2_pass("g2s1h2B", 1, 1, 1, yt2[3])

        # update W-stage + final combine, 4 batches per iter
        for pi in range(B // NB):
            b0 = NB * pi
            kts = load_ktiles(pi, candT_dram.opt(), yt2)
            hc_ps = [psum.tile([H, NOWN], F32, name=f"hcps{b2}", tag="acc",
                               bufs=8) for b2 in range(NB)]
            for k in range(3):
                for b2 in range(NB):
                    nc.tensor.matmul(hc_ps[b2][:], wu_t[:, k, :],
                                     kts[k][:, b2, :],
                                     start=(k == 0), stop=(k == 2))
            hc4 = sb_sm.tile([H, NB, NOWN], BF16, name="hc", tag="zr",
                             bufs=2)
            for b2 in range(NB):
                nc.scalar.activation(hc4[:, b2, :], hc_ps[b2][:], AF.Tanh,
                                     bias=bu_t[:])

            # out = hc + r * (state - hc); state rows are xsT_own[0:H]
            stT4 = sb_sm.tile([H, NB, NOWN], BF16, name="stTu", tag="stg",
                              bufs=2)
            nc.sync.dma_start(
                stT4[:],
                d["xsT_own"].ap()[b0 * C:(b0 + NB) * C, :]
                .rearrange("(b c) n -> c b n", b=NB)[0:H])
            rT4 = sb_sm.tile([H, NB, NOWN], BF16, name="rT", tag="rT",
                             bufs=2)
            nc.sync.dma_start(
                rT4[:],
                rt_dram.opt()[b0:b0 + NB].rearrange("b (h n) -> h b n",
                                                    h=H))
            tmp4 = sb_sm.tile([H, NB, NOWN], BF16, name="tmp", tag="tmp",
                              bufs=2)
            nc.vector.tensor_sub(tmp4[:], stT4[:], hc4[:])
            nc.vector.tensor_mul(tmp4[:], rT4[:], tmp4[:])
            ot4 = sb_sm.tile([H, NB, NOWN], F32, name="ot", tag="ot",
                             bufs=1)
            nc.vector.tensor_add(ot4[:], hc4[:], tmp4[:])
            nc.scalar.dma_start(
                d["outT"].ap()[b0:b0 + NB].rearrange("b h n -> h b n"),
                ot4[:])


def prepare_in_maps(x, state, support0, support1, W_gate, b_gate,
                    W_update, b_update):
    BFNP = mybir.dt.np(BF16)
    xs = np.concatenate([x, state], axis=-1)          # [B, N, C]
    xs_nm = np.ascontiguousarray(
        xs.transpose(1, 0, 2).reshape(N, BC)).astype(BFNP)
    # feature-major input for W / elementwise uses [state(64); x(2)] rows
    sx_nm = np.ascontiguousarray(
        np.concatenate([state, x], axis=-1)
        .transpose(1, 0, 2).reshape(N, BC)).astype(np.float32)
    perm = np.r_[DIN:C, 0:DIN]                 # [x, state] -> [state, x]

    # fold the Chebyshev combination x2 = 2*A@x1 - x0 into W:
    # W0 -= (W2 + W4); W2 *= 2; W4 *= 2  (per 66-row block)
    def fold(W):
        Wf = np.ascontiguousarray(W, dtype=np.float32).copy()
        Wf[0:C] -= Wf[2 * C:3 * C] + Wf[4 * C:5 * C]
        Wf[2 * C:3 * C] *= 2.0
        Wf[4 * C:5 * C] *= 2.0
        return Wf

    Wg_dev = fold(W_gate)
    Wg_dev[0:C] = Wg_dev[0:C][perm]            # only the X-block reads xsT
    Wu_dev = fold(W_update)
    for j in range(5):                         # all of cand's blocks permute
        Wu_dev[j * C:(j + 1) * C] = Wu_dev[j * C:(j + 1) * C][perm]
    Wg_dev = Wg_dev.astype(BFNP)
    Wu_dev = Wu_dev.astype(BFNP)

    xs_main = np.ascontiguousarray(xs_nm[:, :MAIN])
    xs_rag = np.ascontiguousarray(xs_nm[:, MAIN:]).reshape(MT, 128, RAG)
    bg = np.ascontiguousarray(b_gate, dtype=np.float32).reshape(2 * H, 1)
    bu = np.ascontiguousarray(b_update, dtype=np.float32).reshape(H, 1)
    s0b = np.asarray(support0, dtype=np.float32).astype(BFNP)
    s1b = np.asarray(support1, dtype=np.float32).astype(BFNP)

    in_maps = []
    for r in range(NCORES):
        n0 = r * NOWN
        in_maps.append({
            "Ts": np.ascontiguousarray(
                np.stack([s0b[n0:n0 + NOWN, :].T,
                          s1b[n0:n0 + NOWN, :].T])),
            "xs_main": xs_main,
            "xs_rag": xs_rag,
            "xsT_own": np.ascontiguousarray(
                sx_nm[n0:n0 + NOWN].T).astype(BFNP),
            "Wg": Wg_dev, "bg": bg, "Wu": Wu_dev, "bu": bu,
        })
    return in_maps


def assemble_output(results):
    out = np.empty((B, N, H), dtype=np.float32)
    for r in range(NCORES):
        n0 = r * NOWN
        out[:, n0:n0 + NOWN, :] = results[r]["outT"].transpose(0, 2, 1)
    return out


def get_nc():
    if "nc" not in _NC_CACHE:
        _NC_CACHE["nc"] = build_nc()
    return _NC_CACHE["nc"]


def kernel(x, state, support0, support1, W_gate, b_gate, W_update, b_update):
    nc = get_nc()
    in_maps = prepare_in_maps(x, state, support0, support1,
                              W_gate, b_gate, W_update, b_update)
    prev = os.environ.get("BASS_NEVER_TRACE")
    os.environ["BASS_NEVER_TRACE"] = "1"
    try:
        res = run_bass_kernel_spmd(nc, in_maps, list(range(NCORES)),
                                   trace=False)
    finally:
        if prev is None:
            os.environ.pop("BASS_NEVER_TRACE", None)
        else:
            os.environ["BASS_NEVER_TRACE"] = prev
    return assemble_output(res.results)
